# revision 31
# baseline (speedup 1.0000x reference)
# Trainium2 Bass kernel for nn_Block (dense transformer block, single head).
#
# Strategy: pure data-parallel over batch. 32 batches / 8 cores = 4 per core.
# All weights replicated per core; no collectives.
#
# Math (per batch, faithful to reference):
#   h   = LN(x) * w1 + b1            (LN affine folded into qkv weights on host)
#   qkv = h @ qkv_w.T                (q,k channel-major; v token-major)
#   P   = softmax(q k^T / sqrt(C))   (computed as exp(S^T) + PE column-sum denom)
#   y   = P @ v                      (computed channel-major: y^T)
#   y_s = reshape(y^T, [N, C])       (torch transpose+reshape scramble; done via
#                                     a DRAM round-trip: write y^T flat, re-read
#                                     as [N, C] rows)
#   x2  = 2*(y_s @ proj_w.T + proj_b)  (factor 2 folded into proj weights)
#   out = x2 + fc2(gelu(fc1(LN(x2))))
#
# When norm1_b == 0 (always true for this problem's fixed init), q/k are never
# materialized: S = h (Wq^T Wk) h^T with A = Wq^T Wk precomputed on the host.
#
# Matmuls run as float32r (TF32-class mantissa, 4x the fp32 PE rate).

import numpy as np

N_CORES = 8
B = 32
BPC = B // N_CORES  # batches per core
N = 577
C = 768
H = 3072
EPS = 1e-5
NT = 5                     # token tiles of 128 (last has 65 rows)
ROWS = [128, 128, 128, 128, 65]
NC6 = C // 128             # 6 channel tiles
NH24 = H // 128            # 24 hidden tiles
# free-dim chunking of the 577-token axis: both >=256 so f32r runs 1 cyc/row.
# psum layout keeps chunk B in bank 1 (offset 512) so no matmul write
# straddles a 2KB psum bank boundary.
N_PAD = 578                # f32r matmuls need even free sizes; 577 is odd.
CHUNKS = [(0, 290, 0), (290, 288, 512)]   # (src_off, len, psum_off)


def _legalize_sync(nc, mybir):
    """Walrus allows only one sync-wait slot per hardware instruction (fp32/f32r
    matmuls are strictest); hoist excess waits onto InstNoOp carriers inserted
    immediately before, and excess updates onto following nops (never for DMA
    completion updates)."""
    n_fix = 0
    for f in nc.m.functions:
        for bb in f.blocks:
            il = bb.instructions
            out = []
            changed = False
            for inst in il:
                si = inst.sync_info
                waits = list(si.on_wait) if si is not None and si.on_wait else []
                upds = list(si.on_update) if si is not None and si.on_update else []
                if len(waits) > 1:
                    for w in waits[:-1]:
                        out.append(mybir.InstNoOp(
                            name=f"{inst.name}-wn{n_fix}-{len(out)}",
                            sync_info=mybir.SyncInfo(on_wait=[w], on_update=[]),
                            bass_nofuse=True,
                            engine=inst.engine,
                        ))
                    inst.sync_info = mybir.SyncInfo(on_wait=waits[-1:], on_update=upds)
                    changed = True
                    n_fix += 1
                out.append(inst)
                is_dma = isinstance(inst, (mybir.InstDMACopy, mybir.InstDMA,
                                           mybir.InstLoad, mybir.InstSave,
                                           mybir.InstTensorLoad, mybir.InstTensorSave))
                if not is_dma and len(upds) > 2:
                    si2 = inst.sync_info
                    inst.sync_info = mybir.SyncInfo(
                        on_wait=list(si2.on_wait), on_update=upds[:2])
                    for u in upds[2:]:
                        out.append(mybir.InstNoOp(
                            name=f"{inst.name}-un{n_fix}-{len(out)}",
                            sync_info=mybir.SyncInfo(on_wait=[], on_update=[u]),
                            bass_nofuse=True,
                            engine=inst.engine,
                        ))
                    changed = True
                    n_fix += 1
            if changed:
                bb.instructions = out
    return n_fix


def _build_program(has_bv, has_qkb):
    import concourse.bass as bass
    import concourse.mybir as mybir
    from concourse.tile import TileContext
    from concourse.masks import make_identity

    f32 = mybir.dt.float32
    f32r = mybir.dt.float32r
    AF = mybir.ActivationFunctionType
    ALU = mybir.AluOpType

    nc = bass.Bass()

    # --- DRAM parameters (per core) ---
    x_in = nc.declare_dram_parameter("x", [BPC, N, C], f32, isOutput=False)
    wq = nc.declare_dram_parameter("wq", [C, C], f32r, isOutput=False)
    wk = nc.declare_dram_parameter("wk", [C, C], f32r, isOutput=False)
    wv = nc.declare_dram_parameter("wv", [C, C], f32r, isOutput=False)
    pw2T = nc.declare_dram_parameter("pw2T", [C, C], f32r, isOutput=False)
    fc1wT = nc.declare_dram_parameter("fc1wT", [C, H], f32r, isOutput=False)
    fc2wT = nc.declare_dram_parameter("fc2wT", [H, C], f32r, isOutput=False)
    bq2d = nc.declare_dram_parameter("bq2d", [NC6, 128], f32, isOutput=False)
    bk2d = nc.declare_dram_parameter("bk2d", [NC6, 128], f32, isOutput=False)
    fc1b2d = nc.declare_dram_parameter("fc1b2d", [NH24, 128], f32, isOutput=False)
    pb2 = nc.declare_dram_parameter("pb2", [C], f32, isOutput=False)
    fc2b = nc.declare_dram_parameter("fc2b", [C], f32, isOutput=False)
    bv1d = nc.declare_dram_parameter("bv1d", [C], f32, isOutput=False)
    out_d = nc.declare_dram_parameter("out", [BPC, N, C], f32, isOutput=True)
    tattn_d = nc.declare_dram_parameter("tattn", [BPC, N - 1], f32, isOutput=True)

    scale = float(C) ** -0.5

    def bcast_ap(param, n):
        ap = param[:]
        return bass.AP(tensor=ap.tensor, offset=0, ap=[[0, 128], [1, n]])

    def t_ap(param, rows, cols):
        # [rows, cols] dram -> [cols(part), rows(free)] transposed load AP
        ap = param[:]
        return bass.AP(tensor=ap.tensor, offset=0, ap=[[1, cols], [cols, rows]])

    with TileContext(nc) as tc:
        with (
            tc.tile_pool(name="consts", bufs=1) as consts,
            tc.tile_pool(name="small", bufs=10) as small,
            tc.tile_pool(name="row577", bufs=2) as row577,
            tc.tile_pool(name="p577", bufs=20) as p577,
            tc.tile_pool(name="p768", bufs=12) as p768,
            tc.tile_pool(name="xp", bufs=6) as xp,
            tc.tile_pool(name="wchunk", bufs=6) as wchunk,
            tc.tile_pool(name="fc2c", bufs=8) as fc2c_pool,
            tc.tile_pool(name="wvp", bufs=1) as wvp,
            tc.tile_pool(name="ps", bufs=4, space="PSUM") as ps,
            tc.tile_pool(name="dram", bufs=2, space="DRAM") as dpool,
        ):
            # --- constants ---
            ident = consts.tile([128, 128], f32)
            make_identity(nc, ident)
            ident_r = consts.tile([128, 128], f32r)
            nc.scalar.copy(ident_r, ident)
            ones_f = consts.tile([128, 1], f32)
            nc.vector.memset(ones_f, 1.0)
            ones_col = consts.tile([128, 1], f32r)
            nc.scalar.copy(ones_col, ones_f)
            ones_row = consts.tile([1, 128], f32)
            nc.vector.memset(ones_row, 1.0)
            eps_t = consts.tile([128, 1], f32)
            nc.vector.memset(eps_t, EPS)
            bq_sb = consts.tile([128, NC6], f32)
            nc.sync.dma_start(out=bq_sb, in_=t_ap(bq2d, NC6, 128))
            bk_sb = consts.tile([128, NC6], f32)
            nc.sync.dma_start(out=bk_sb, in_=t_ap(bk2d, NC6, 128))
            fc1b_sb = consts.tile([128, NH24], f32)
            nc.sync.dma_start(out=fc1b_sb, in_=t_ap(fc1b2d, NH24, 128))
            pb2_bc = consts.tile([128, C], f32)
            nc.sync.dma_start(out=pb2_bc, in_=bcast_ap(pb2, C))
            fc2b_bc = consts.tile([128, C], f32)
            nc.sync.dma_start(out=fc2b_bc, in_=bcast_ap(fc2b, C))
            if has_bv:
                bv_bc = consts.tile([128, C], f32)
                nc.sync.dma_start(out=bv_bc, in_=bcast_ap(bv1d, C))
            pw2T_sb = consts.tile([128, NC6, C], f32r)

            def load_x(b):
                tiles = []
                for t in range(NT):
                    r = ROWS[t]
                    xt = xp.tile([128, C], f32, tag="xt")
                    nc.gpsimd.dma_start(out=xt[:r], in_=x_in[b, t * 128:t * 128 + r, :])
                    tiles.append(xt)
                return tiles

            def load_wq():
                tiles = []
                for j in range(NC6):
                    wc = wchunk.tile([128, NC6, 128], f32r, tag="ws")
                    nc.sync.dma_start(
                        out=wc,
                        in_=wq[:, j * 128:(j + 1) * 128]
                        .rearrange("(a p) j -> p a j", p=128))
                    tiles.append(wc)
                return tiles

            xt_next = load_x(0)
            wq_next = None
            for b in range(BPC):
                # ---------------- Phase A: LN1 + transpose -> hT ----------------
                xts = xt_next
                ht = []
                for t in range(NT):
                    r = ROWS[t]
                    xt = xts[t]
                    st = small.tile([128, 2, 6], f32, tag="bn")
                    xg = xt.rearrange("p (g d) -> p g d", g=2)
                    nc.vector.bn_stats(out=st[:r, 0], in_=xg[:r, 0])
                    nc.vector.bn_stats(out=st[:r, 1], in_=xg[:r, 1])
                    mv = small.tile([128, 2], f32, tag="mv")
                    nc.vector.bn_aggr(out=mv[:r], in_=st[:r])
                    sd = small.tile([128, 1], f32, tag="sd")
                    nc.scalar.activation(sd[:r], mv[:r, 1:2], AF.Sqrt, bias=eps_t[:r])
                    rc = small.tile([128, 1], f32, tag="rc")
                    nc.vector.reciprocal(rc[:r], sd[:r])
                    h = p768.tile([128, C], f32r, tag="a768")
                    nc.vector.tensor_scalar(
                        h[:r], xt[:r], mv[:r, 0:1], rc[:r], ALU.subtract, ALU.mult)
                    ht.append(h)
                hT = []
                for c in range(NC6):
                    pA = ps.tile([128, 1024], f32r, tag="ps")
                    for t in range(NT):
                        r = ROWS[t] if ROWS[t] % 2 == 0 else ROWS[t] + 1
                        nc.tensor.transpose(
                            pA[:, t * 128:t * 128 + r],
                            ht[t][:r, c * 128:(c + 1) * 128], ident_r[:r, :r])
                    hc = p577.tile([128, N_PAD], f32r, tag="a577")
                    nc.scalar.copy(hc, pA[:, :N_PAD])
                    hT.append(hc)

                # ---------------- Phase B: qkv ----------------
                qT, kT = [], []
                wsrcs = ((wq, bq_sb, qT), (wk, bk_sb, kT)) if has_qkb \
                    else ((wq, bq_sb, qT),)
                for w_par, b_sb, dst in wsrcs:
                    for j in range(NC6):
                        if w_par is wq and wq_next is not None:
                            wc = wq_next[j]
                        else:
                            wc = wchunk.tile([128, NC6, 128], f32r, tag="ws")
                            nc.sync.dma_start(
                                out=wc,
                                in_=w_par[:, j * 128:(j + 1) * 128]
                                .rearrange("(a p) j -> p a j", p=128))
                        pQ = ps.tile([128, 1024], f32, tag="ps")
                        for c in range(NC6):
                            for (so, ln, po) in CHUNKS:
                                nc.tensor.matmul(
                                    pQ[:, po:po + ln], wc[:, c, :],
                                    hT[c][:, so:so + ln],
                                    start=(c == 0), stop=(c == NC6 - 1))
                        qj = p577.tile([128, N_PAD], f32r, tag="a577")
                        for (so, ln, po) in CHUNKS:
                            nc.scalar.activation(
                                qj[:, so:so + ln], pQ[:, po:po + ln],
                                AF.Identity, bias=b_sb[:, j:j + 1])
                        dst.append(qj)
                wv_sb = wvp.tile([128, NC6, C], f32r, tag="wv")
                for _c in range(NC6):
                    nc.sync.dma_start(
                        out=wv_sb[:, _c],
                        in_=wv[_c * 128:(_c + 1) * 128, :].bitcast(f32r))
                v = []
                for t in range(NT):
                    r = ROWS[t]
                    pV = ps.tile([128, 1024], f32, tag="ps")
                    for c in range(NC6):
                        nc.tensor.matmul(pV[:r, 0:512],
                                         hT[c][:, t * 128:t * 128 + r],
                                         wv_sb[:, c, 0:512],
                                         start=(c == 0), stop=(c == NC6 - 1))
                        nc.tensor.matmul(pV[:r, 512:768],
                                         hT[c][:, t * 128:t * 128 + r],
                                         wv_sb[:, c, 512:768],
                                         start=(c == 0), stop=(c == NC6 - 1))
                    vt = p768.tile([128, C], f32r, tag="a768")
                    if has_bv:
                        nc.vector.tensor_tensor(
                            out=vt[:r], in0=pV[:r, 0:768], in1=bv_bc[:r], op=ALU.add)
                    else:
                        nc.scalar.copy(vt[:r], pV[:r, 0:768])
                    v.append(vt)

                # ---------------- Phase C: S^T, exp, denom, r ----------------
                kTS = kT if has_qkb else hT
                PuT = []
                for m in range(NT):
                    rm = ROWS[m]
                    pS = ps.tile([128, 1024], f32, tag="ps")
                    for c in range(NC6):
                        for (so, ln, po) in CHUNKS:
                            nc.tensor.matmul(
                                pS[:rm, po:po + ln],
                                kTS[c][:, m * 128:m * 128 + rm],
                                qT[c][:, so:so + ln],
                                start=(c == 0), stop=(c == NC6 - 1))
                    pu = p577.tile([128, N_PAD], f32r, tag="a577")
                    for (so, ln, po) in CHUNKS:
                        nc.scalar.activation(
                            pu[:rm, so:so + ln], pS[:rm, po:po + ln],
                            AF.Exp, scale=scale)
                    PuT.append(pu)
                pD = ps.tile([128, 1024], f32, tag="ps")
                for m in range(NT):
                    rm = ROWS[m]
                    for (so, ln, po) in CHUNKS:
                        nc.tensor.matmul(
                            pD[0:1, po:po + ln], ones_col[:rm],
                            PuT[m][:rm, so:so + ln],
                            start=(m == 0), stop=(m == NT - 1))
                dsum = row577.tile([1, N_PAD], f32, tag="row")
                for (so, ln, po) in CHUNKS:
                    nc.scalar.copy(dsum[:, so:so + ln], pD[0:1, po:po + ln])
                rinv = row577.tile([1, N_PAD], f32, tag="row")
                nc.vector.reciprocal(rinv, dsum)
                pR = ps.tile([128, 1024], f32, tag="ps")
                for (so, ln, po) in CHUNKS:
                    nc.tensor.matmul(pR[:, po:po + ln], ones_row,
                                     rinv[:, so:so + ln], start=True, stop=True)
                rbc = p577.tile([128, N_PAD], f32, tag="a577")
                for (so, ln, po) in CHUNKS:
                    nc.scalar.copy(rbc[:, so:so + ln], pR[:, po:po + ln])

                # token_attn: normalized attention row 0 = PuT[m][:, 0] * r[0]
                ta = small.tile([128, NT], f32, tag="ta")
                for m in range(NT):
                    nc.vector.tensor_copy(ta[:ROWS[m], m:m + 1], PuT[m][:ROWS[m], 0:1])
                nc.vector.tensor_scalar_mul(ta, ta, rbc[:, 0:1])
                nc.sync.dma_start(
                    out=bass.AP(tensor=tattn_d[:].tensor, offset=b * (N - 1),
                                ap=[[1, 127]]),
                    in_=ta[1:128, 0:1])
                nc.sync.dma_start(
                    out=bass.AP(tensor=tattn_d[:].tensor, offset=b * (N - 1) + 127,
                                ap=[[1, 128], [128, 3]]),
                    in_=ta[:, 1:4])
                nc.sync.dma_start(
                    out=bass.AP(tensor=tattn_d[:].tensor, offset=b * (N - 1) + 511,
                                ap=[[1, 65]]),
                    in_=ta[0:65, 4:5])

                if b == 0:
                    nc.sync.dma_start(
                        out=pw2T_sb,
                        in_=pw2T[:, :].rearrange("(a p) o -> p a o", p=128))
                # ---------------- Phase D: y^T = (v^T @ Pu^T) * r ----------------
                ydram = dpool.tile([C * N], f32, tag="yd")
                yd_w = ydram.rearrange("(a b) -> a b", b=N)    # [C, N] write view
                yd_r = ydram.rearrange("(a b) -> a b", b=C)    # [N, C] read view
                for c in range(NC6):
                    pY = ps.tile([128, 1024], f32, tag="ps")
                    for m in range(NT):
                        rm = ROWS[m]
                        for (so, ln, po) in CHUNKS:
                            nc.tensor.matmul(
                                pY[:, po:po + ln],
                                v[m][:rm, c * 128:(c + 1) * 128],
                                PuT[m][:rm, so:so + ln],
                                start=(m == 0), stop=(m == NT - 1))
                    yc = p577.tile([128, N_PAD], f32, tag="a577")
                    for (so, ln, po) in CHUNKS:
                        nc.vector.tensor_tensor(
                            out=yc[:, so:so + ln], in0=pY[:, po:po + ln],
                            in1=rbc[:, so:so + ln], op=ALU.mult)
                    nc.sync.dma_start(out=yd_w[c * 128:(c + 1) * 128, :], in_=yc[:, :N])

                # ---------------- Phase E: scramble read, transpose, proj ----------------
                ys = []
                for t in range(NT):
                    r = ROWS[t]
                    yst = p768.tile([128, C], f32r, tag="a768")
                    nc.sync.dma_start(out=yst[:r],
                                      in_=yd_r[t * 128:t * 128 + r, :].bitcast(f32r))
                    ys.append(yst)
                ysT = []
                for c in range(NC6):
                    pT2 = ps.tile([128, 1024], f32r, tag="ps")
                    for t in range(NT):
                        r = ROWS[t] if ROWS[t] % 2 == 0 else ROWS[t] + 1
                        nc.tensor.transpose(
                            pT2[:, t * 128:t * 128 + r],
                            ys[t][:r, c * 128:(c + 1) * 128], ident_r[:r, :r])
                    yTc = p577.tile([128, N_PAD], f32r, tag="a577")
                    nc.scalar.copy(yTc, pT2[:, :N_PAD])
                    ysT.append(yTc)
                x2 = []
                for t in range(NT):
                    r = ROWS[t]
                    pP = ps.tile([128, 1024], f32, tag="ps")
                    for c in range(NC6):
                        nc.tensor.matmul(pP[:r, 0:512],
                                         ysT[c][:, t * 128:t * 128 + r],
                                         pw2T_sb[:, c, 0:512],
                                         start=(c == 0), stop=(c == NC6 - 1))
                        nc.tensor.matmul(pP[:r, 512:768],
                                         ysT[c][:, t * 128:t * 128 + r],
                                         pw2T_sb[:, c, 512:768],
                                         start=(c == 0), stop=(c == NC6 - 1))
                    x2t = p768.tile([128, C], f32, tag="a768")
                    nc.vector.tensor_tensor(
                        out=x2t[:r], in0=pP[:r, 0:768], in1=pb2_bc[:r], op=ALU.add)
                    x2.append(x2t)

                # ---------------- Phase F: LN2 + transpose -> mT ----------------
                mtk = []
                for t in range(NT):
                    r = ROWS[t]
                    st = small.tile([128, 2, 6], f32, tag="bn")
                    xg = x2[t].rearrange("p (g d) -> p g d", g=2)
                    nc.vector.bn_stats(out=st[:r, 0], in_=xg[:r, 0])
                    nc.vector.bn_stats(out=st[:r, 1], in_=xg[:r, 1])
                    mv = small.tile([128, 2], f32, tag="mv")
                    nc.vector.bn_aggr(out=mv[:r], in_=st[:r])
                    sd = small.tile([128, 1], f32, tag="sd")
                    nc.scalar.activation(sd[:r], mv[:r, 1:2], AF.Sqrt, bias=eps_t[:r])
                    rc = small.tile([128, 1], f32, tag="rc")
                    nc.vector.reciprocal(rc[:r], sd[:r])
                    mt = p768.tile([128, C], f32r, tag="a768")
                    nc.vector.tensor_scalar(
                        mt[:r], x2[t][:r], mv[:r, 0:1], rc[:r], ALU.subtract, ALU.mult)
                    mtk.append(mt)
                mT = []
                for c in range(NC6):
                    pT3 = ps.tile([128, 1024], f32r, tag="ps")
                    for t in range(NT):
                        r = ROWS[t] if ROWS[t] % 2 == 0 else ROWS[t] + 1
                        nc.tensor.transpose(
                            pT3[:, t * 128:t * 128 + r],
                            mtk[t][:r, c * 128:(c + 1) * 128], ident_r[:r, :r])
                    mc = p577.tile([128, N_PAD], f32r, tag="a577")
                    nc.scalar.copy(mc, pT3[:, :N_PAD])
                    mT.append(mc)

                # ---------------- Phase G: fc1 (hf quarters) + fc2 ----------------
                if b + 1 < BPC:
                    xt_next = load_x(b + 1)
                g_sb = [None] * NT
                NGRP = 4
                GSZ = NH24 // NGRP
                for half in range(NGRP):
                    fT = []
                    for hl in range(GSZ):
                        hh = half * GSZ + hl
                        f1c = wchunk.tile([128, NC6, 128], f32r, tag="ws")
                        nc.sync.dma_start(
                            out=f1c,
                            in_=fc1wT[:, hh * 128:(hh + 1) * 128]
                            .rearrange("(a p) j -> p a j", p=128))
                        pF = ps.tile([128, 1024], f32, tag="ps")
                        for c in range(NC6):
                            for (so, ln, po) in CHUNKS:
                                nc.tensor.matmul(
                                    pF[:, po:po + ln], f1c[:, c, :],
                                    mT[c][:, so:so + ln],
                                    start=(c == 0), stop=(c == NC6 - 1))
                        ft = p577.tile([128, N_PAD], f32r, tag="a577")
                        for (so, ln, po) in CHUNKS:
                            nc.scalar.activation(
                                ft[:, so:so + ln], pF[:, po:po + ln],
                                AF.Gelu, bias=fc1b_sb[:, hh:hh + 1])
                        fT.append(ft)
                    f2c = []
                    for hl in range(GSZ):
                        hh = half * GSZ + hl
                        fc = fc2c_pool.tile([128, C], f32r, tag="f2c")
                        nc.sync.dma_start(out=fc, in_=fc2wT[hh * 128:(hh + 1) * 128, :])
                        f2c.append(fc)
                    if half == NGRP - 1:
                        wq_next = load_wq() if b + 1 < BPC else None
                    for t in range(NT):
                        r = ROWS[t]
                        pG = ps.tile([128, 1024], f32, tag="ps")
                        for hl in range(GSZ):
                            nc.tensor.matmul(pG[:r, 0:512],
                                             fT[hl][:, t * 128:t * 128 + r],
                                             f2c[hl][:, 0:512],
                                             start=(hl == 0), stop=(hl == GSZ - 1))
                            nc.tensor.matmul(pG[:r, 512:768],
                                             fT[hl][:, t * 128:t * 128 + r],
                                             f2c[hl][:, 512:768],
                                             start=(hl == 0), stop=(hl == GSZ - 1))
                        if half == 0:
                            gt = p768.tile([128, C], f32, tag="a768")
                            nc.vector.tensor_tensor(
                                out=gt[:r], in0=pG[:r, 0:768], in1=fc2b_bc[:r],
                                op=ALU.add)
                            g_sb[t] = gt
                        elif half < NGRP - 1:
                            nc.vector.tensor_tensor(
                                out=g_sb[t][:r], in0=pG[:r, 0:768],
                                in1=g_sb[t][:r], op=ALU.add)
                        else:
                            nc.vector.tensor_tensor(
                                out=g_sb[t][:r], in0=pG[:r, 0:768],
                                in1=g_sb[t][:r], op=ALU.add)
                            ot = p768.tile([128, C], f32, tag="a768")
                            nc.vector.tensor_tensor(
                                out=ot[:r], in0=g_sb[t][:r], in1=x2[t][:r],
                                op=ALU.add)
                            nc.sync.dma_start(
                                out=out_d[b, t * 128:t * 128 + r, :], in_=ot[:r])

    _legalize_sync(nc, mybir)
    return nc


_PROG_CACHE = {}


def _get_program(has_bv, has_qkb):
    key = (has_bv, has_qkb)
    if key not in _PROG_CACHE:
        _PROG_CACHE[key] = _build_program(has_bv, has_qkb)
    return _PROG_CACHE[key]


def _prep_host(x, norm1_w, norm1_b, qkv_w, proj_w, proj_b,
               norm2_w, norm2_b, fc1_w, fc1_b, fc2_w, fc2_b):
    f = np.float32
    x = np.ascontiguousarray(np.asarray(x, f))
    w1 = np.asarray(norm1_w, f); b1 = np.asarray(norm1_b, f)
    w2 = np.asarray(norm2_w, f); b2 = np.asarray(norm2_b, f)
    qkv_w = np.asarray(qkv_w, f); proj_w = np.asarray(proj_w, f)
    fc1_w = np.asarray(fc1_w, f); fc2_w = np.asarray(fc2_w, f)

    qkv_w_eff = qkv_w * w1[None, :]
    qkv_b_eff = qkv_w @ b1
    wqT = np.ascontiguousarray(qkv_w_eff[0:C].T)
    wkT = np.ascontiguousarray(qkv_w_eff[C:2 * C].T)
    wvT = np.ascontiguousarray(qkv_w_eff[2 * C:3 * C].T)
    bq = np.ascontiguousarray(qkv_b_eff[0:C].reshape(NC6, 128))
    bk = np.ascontiguousarray(qkv_b_eff[C:2 * C].reshape(NC6, 128))
    bv = np.ascontiguousarray(qkv_b_eff[2 * C:3 * C])
    pw2T = np.ascontiguousarray((2.0 * proj_w).T)
    pb2 = np.ascontiguousarray(2.0 * np.asarray(proj_b, f))
    fc1wT = np.ascontiguousarray((fc1_w * w2[None, :]).T)
    fc1b_eff = np.ascontiguousarray(
        (np.asarray(fc1_b, f) + fc1_w @ b2).reshape(NH24, 128))
    fc2wT = np.ascontiguousarray(fc2_w.T)
    fc2b = np.ascontiguousarray(np.asarray(fc2_b, f))
    return x, wqT, wkT, wvT, bq, bk, bv, pw2T, pb2, fc1wT, fc1b_eff, fc2wT, fc2b


def _run(inputs, trace=False):
    from concourse.bass_utils import run_bass_kernel_spmd

    (x, wqT, wkT, wvT, bq, bk, bv, pw2T, pb2, fc1wT, fc1b_eff,
     fc2wT, fc2b) = _prep_host(**inputs)
    has_bv = bool(np.any(bv))
    has_qkb = bool(np.any(bq)) or bool(np.any(bk))
    if not has_qkb:
        # S = q k^T = h (Wq^T Wk) h^T: stream A = Wq^T Wk instead of Wq, Wk
        # (wqT/wkT here are already transposed: wqT = Wq^T as [c, j]).
        A = (wqT.astype(np.float64) @ wkT.T.astype(np.float64)).astype(np.float32)
        wqT = np.ascontiguousarray(A)
    nc = _get_program(has_bv, has_qkb)

    in_maps = []
    for core in range(N_CORES):
        in_maps.append({
            "x": np.ascontiguousarray(x[core * BPC:(core + 1) * BPC]),
            "wq": wqT, "wk": wkT, "wv": wvT, "pw2T": pw2T,
            "fc1wT": fc1wT, "fc2wT": fc2wT,
            "bq2d": bq, "bk2d": bk, "fc1b2d": fc1b_eff,
            "pb2": pb2, "fc2b": fc2b, "bv1d": bv,
        })
    res = run_bass_kernel_spmd(nc, in_maps, core_ids=list(range(N_CORES)),
                               trace=trace)
    out = np.concatenate([res.results[i]["out"] for i in range(N_CORES)], axis=0)
    ta = np.concatenate([res.results[i]["tattn"] for i in range(N_CORES)], axis=0)
    asz = int((N - 1) ** 0.5)
    token_attn = ta.reshape(B, 1, asz, asz).astype(np.float32)
    return (out.astype(np.float32), token_attn), res


def kernel(**inputs):
    (out, token_attn), _ = _run(inputs, trace=False)
    return (out, token_attn)


# revision 32
# speedup vs baseline: 1.0090x; 1.0090x over previous
# Trainium2 Bass kernel for nn_Block (dense transformer block, single head).
#
# Strategy: pure data-parallel over batch. 32 batches / 8 cores = 4 per core.
# All weights replicated per core; no collectives.
#
# Math (per batch, faithful to reference):
#   h   = LN(x) * w1 + b1            (LN affine folded into qkv weights on host)
#   qkv = h @ qkv_w.T                (q,k channel-major; v token-major)
#   P   = softmax(q k^T / sqrt(C))   (computed as exp(S^T) + PE column-sum denom)
#   y   = P @ v                      (computed channel-major: y^T)
#   y_s = reshape(y^T, [N, C])       (torch transpose+reshape scramble; done via
#                                     a DRAM round-trip: write y^T flat, re-read
#                                     as [N, C] rows)
#   x2  = 2*(y_s @ proj_w.T + proj_b)  (factor 2 folded into proj weights)
#   out = x2 + fc2(gelu(fc1(LN(x2))))
#
# When norm1_b == 0 (always true for this problem's fixed init), q/k are never
# materialized: S = h (Wq^T Wk) h^T with A = Wq^T Wk precomputed on the host.
#
# Matmuls run as float32r (TF32-class mantissa, 4x the fp32 PE rate).

import numpy as np

N_CORES = 8
B = 32
BPC = B // N_CORES  # batches per core
N = 577
C = 768
H = 3072
EPS = 1e-5
NT = 5                     # token tiles of 128 (last has 65 rows)
ROWS = [128, 128, 128, 128, 65]
NC6 = C // 128             # 6 channel tiles
NH24 = H // 128            # 24 hidden tiles
# free-dim chunking of the 577-token axis: both >=256 so f32r runs 1 cyc/row.
# psum layout keeps chunk B in bank 1 (offset 512) so no matmul write
# straddles a 2KB psum bank boundary.
N_PAD = 578                # f32r matmuls need even free sizes; 577 is odd.
CHUNKS = [(0, 290, 0), (290, 288, 512)]   # (src_off, len, psum_off)


def _legalize_sync(nc, mybir):
    """Walrus allows only one sync-wait slot per hardware instruction (fp32/f32r
    matmuls are strictest); hoist excess waits onto InstNoOp carriers inserted
    immediately before, and excess updates onto following nops (never for DMA
    completion updates)."""
    n_fix = 0
    for f in nc.m.functions:
        for bb in f.blocks:
            il = bb.instructions
            out = []
            changed = False
            for inst in il:
                si = inst.sync_info
                waits = list(si.on_wait) if si is not None and si.on_wait else []
                upds = list(si.on_update) if si is not None and si.on_update else []
                if len(waits) > 1:
                    for w in waits[:-1]:
                        out.append(mybir.InstNoOp(
                            name=f"{inst.name}-wn{n_fix}-{len(out)}",
                            sync_info=mybir.SyncInfo(on_wait=[w], on_update=[]),
                            bass_nofuse=True,
                            engine=inst.engine,
                        ))
                    inst.sync_info = mybir.SyncInfo(on_wait=waits[-1:], on_update=upds)
                    changed = True
                    n_fix += 1
                out.append(inst)
                is_dma = isinstance(inst, (mybir.InstDMACopy, mybir.InstDMA,
                                           mybir.InstLoad, mybir.InstSave,
                                           mybir.InstTensorLoad, mybir.InstTensorSave))
                if not is_dma and len(upds) > 2:
                    si2 = inst.sync_info
                    inst.sync_info = mybir.SyncInfo(
                        on_wait=list(si2.on_wait), on_update=upds[:2])
                    for u in upds[2:]:
                        out.append(mybir.InstNoOp(
                            name=f"{inst.name}-un{n_fix}-{len(out)}",
                            sync_info=mybir.SyncInfo(on_wait=[], on_update=[u]),
                            bass_nofuse=True,
                            engine=inst.engine,
                        ))
                    changed = True
                    n_fix += 1
            if changed:
                bb.instructions = out
    return n_fix


def _build_program(has_bv, has_qkb):
    import concourse.bass as bass
    import concourse.mybir as mybir
    from concourse.tile import TileContext
    from concourse.masks import make_identity

    f32 = mybir.dt.float32
    f32r = mybir.dt.float32r
    AF = mybir.ActivationFunctionType
    ALU = mybir.AluOpType

    nc = bass.Bass()

    # --- DRAM parameters (per core) ---
    x_in = nc.declare_dram_parameter("x", [BPC, N, C], f32, isOutput=False)
    wq = nc.declare_dram_parameter("wq", [C, C], f32r, isOutput=False)
    wk = nc.declare_dram_parameter("wk", [C, C], f32r, isOutput=False)
    wv = nc.declare_dram_parameter("wv", [C, C], f32r, isOutput=False)
    pw2T = nc.declare_dram_parameter("pw2T", [C, C], f32r, isOutput=False)
    fc1wT = nc.declare_dram_parameter("fc1wT", [C, H], f32r, isOutput=False)
    fc2wT = nc.declare_dram_parameter("fc2wT", [H, C], f32r, isOutput=False)
    bq2d = nc.declare_dram_parameter("bq2d", [NC6, 128], f32, isOutput=False)
    bk2d = nc.declare_dram_parameter("bk2d", [NC6, 128], f32, isOutput=False)
    fc1b2d = nc.declare_dram_parameter("fc1b2d", [NH24, 128], f32, isOutput=False)
    pb2 = nc.declare_dram_parameter("pb2", [C], f32, isOutput=False)
    fc2b = nc.declare_dram_parameter("fc2b", [C], f32, isOutput=False)
    bv1d = nc.declare_dram_parameter("bv1d", [C], f32, isOutput=False)
    out_d = nc.declare_dram_parameter("out", [BPC, N, C], f32, isOutput=True)
    tattn_d = nc.declare_dram_parameter("tattn", [BPC, N - 1], f32, isOutput=True)

    scale = float(C) ** -0.5

    def bcast_ap(param, n):
        ap = param[:]
        return bass.AP(tensor=ap.tensor, offset=0, ap=[[0, 128], [1, n]])

    def t_ap(param, rows, cols):
        # [rows, cols] dram -> [cols(part), rows(free)] transposed load AP
        ap = param[:]
        return bass.AP(tensor=ap.tensor, offset=0, ap=[[1, cols], [cols, rows]])

    with TileContext(nc) as tc:
        with (
            tc.tile_pool(name="consts", bufs=1) as consts,
            tc.tile_pool(name="small", bufs=10) as small,
            tc.tile_pool(name="row577", bufs=2) as row577,
            tc.tile_pool(name="p577", bufs=20) as p577,
            tc.tile_pool(name="p768", bufs=12) as p768,
            tc.tile_pool(name="xp", bufs=6) as xp,
            tc.tile_pool(name="wchunk", bufs=6) as wchunk,
            tc.tile_pool(name="fc2c", bufs=8) as fc2c_pool,
            tc.tile_pool(name="wvp", bufs=1) as wvp,
            tc.tile_pool(name="ps", bufs=4, space="PSUM") as ps,
            tc.tile_pool(name="dram", bufs=2, space="DRAM") as dpool,
        ):
            # --- constants ---
            ident = consts.tile([128, 128], f32)
            make_identity(nc, ident)
            ident_r = consts.tile([128, 128], f32r)
            nc.scalar.copy(ident_r, ident)
            ones_f = consts.tile([128, 1], f32)
            nc.vector.memset(ones_f, 1.0)
            ones_col = consts.tile([128, 1], f32r)
            nc.scalar.copy(ones_col, ones_f)
            ones_row = consts.tile([1, 128], f32)
            nc.vector.memset(ones_row, 1.0)
            eps_t = consts.tile([128, 1], f32)
            nc.vector.memset(eps_t, EPS)
            bq_sb = consts.tile([128, NC6], f32)
            nc.sync.dma_start(out=bq_sb, in_=t_ap(bq2d, NC6, 128))
            bk_sb = consts.tile([128, NC6], f32)
            nc.sync.dma_start(out=bk_sb, in_=t_ap(bk2d, NC6, 128))
            fc1b_sb = consts.tile([128, NH24], f32)
            nc.sync.dma_start(out=fc1b_sb, in_=t_ap(fc1b2d, NH24, 128))
            pb2_bc = consts.tile([128, C], f32)
            nc.sync.dma_start(out=pb2_bc, in_=bcast_ap(pb2, C))
            fc2b_bc = consts.tile([128, C], f32)
            nc.sync.dma_start(out=fc2b_bc, in_=bcast_ap(fc2b, C))
            if has_bv:
                bv_bc = consts.tile([128, C], f32)
                nc.sync.dma_start(out=bv_bc, in_=bcast_ap(bv1d, C))
            pw2T_sb = consts.tile([128, NC6, C], f32r)

            def load_x(b):
                tiles = []
                for t in range(NT):
                    r = ROWS[t]
                    xt = xp.tile([128, C], f32, tag="xt")
                    nc.gpsimd.dma_start(out=xt[:r], in_=x_in[b, t * 128:t * 128 + r, :])
                    tiles.append(xt)
                return tiles

            def load_wq():
                tiles = []
                for j in range(NC6):
                    wc = wchunk.tile([128, NC6, 128], f32r, tag="ws")
                    nc.sync.dma_start(
                        out=wc,
                        in_=wq[:, j * 128:(j + 1) * 128]
                        .rearrange("(a p) j -> p a j", p=128))
                    tiles.append(wc)
                return tiles

            xt_next = load_x(0)
            wq_next = None
            for b in range(BPC):
                # ---------------- Phase A: LN1 + transpose -> hT ----------------
                xts = xt_next
                ht = []
                for t in range(NT):
                    r = ROWS[t]
                    xt = xts[t]
                    st = small.tile([128, 2, 6], f32, tag="bn")
                    xg = xt.rearrange("p (g d) -> p g d", g=2)
                    nc.vector.bn_stats(out=st[:r, 0], in_=xg[:r, 0])
                    nc.vector.bn_stats(out=st[:r, 1], in_=xg[:r, 1])
                    mv = small.tile([128, 2], f32, tag="mv")
                    nc.vector.bn_aggr(out=mv[:r], in_=st[:r])
                    sd = small.tile([128, 1], f32, tag="sd")
                    nc.scalar.activation(sd[:r], mv[:r, 1:2], AF.Sqrt, bias=eps_t[:r])
                    rc = small.tile([128, 1], f32, tag="rc")
                    nc.vector.reciprocal(rc[:r], sd[:r])
                    nmr = small.tile([128, 1], f32, tag="nmr")
                    nc.vector.tensor_scalar(
                        nmr[:r], mv[:r, 0:1], rc[:r], -1.0, ALU.mult, ALU.mult)
                    h = p768.tile([128, C], f32r, tag="a768")
                    nc.scalar.activation(h[:r], xt[:r], AF.Identity,
                                         bias=nmr[:r], scale=rc[:r])
                    ht.append(h)
                hT = []
                for c in range(NC6):
                    pA = ps.tile([128, 1024], f32r, tag="ps")
                    for t in range(NT):
                        r = ROWS[t] if ROWS[t] % 2 == 0 else ROWS[t] + 1
                        nc.tensor.transpose(
                            pA[:, t * 128:t * 128 + r],
                            ht[t][:r, c * 128:(c + 1) * 128], ident_r[:r, :r])
                    hc = p577.tile([128, N_PAD], f32r, tag="a577")
                    nc.scalar.copy(hc, pA[:, :N_PAD])
                    hT.append(hc)

                # ---------------- Phase B: qkv ----------------
                qT, kT = [], []
                wsrcs = ((wq, bq_sb, qT), (wk, bk_sb, kT)) if has_qkb \
                    else ((wq, bq_sb, qT),)
                for w_par, b_sb, dst in wsrcs:
                    for j in range(NC6):
                        if w_par is wq and wq_next is not None:
                            wc = wq_next[j]
                        else:
                            wc = wchunk.tile([128, NC6, 128], f32r, tag="ws")
                            nc.sync.dma_start(
                                out=wc,
                                in_=w_par[:, j * 128:(j + 1) * 128]
                                .rearrange("(a p) j -> p a j", p=128))
                        pQ = ps.tile([128, 1024], f32, tag="ps")
                        for c in range(NC6):
                            for (so, ln, po) in CHUNKS:
                                nc.tensor.matmul(
                                    pQ[:, po:po + ln], wc[:, c, :],
                                    hT[c][:, so:so + ln],
                                    start=(c == 0), stop=(c == NC6 - 1))
                        qj = p577.tile([128, N_PAD], f32r, tag="a577")
                        for (so, ln, po) in CHUNKS:
                            nc.scalar.activation(
                                qj[:, so:so + ln], pQ[:, po:po + ln],
                                AF.Identity, bias=b_sb[:, j:j + 1])
                        dst.append(qj)
                wv_sb = wvp.tile([128, NC6, C], f32r, tag="wv")
                for _c in range(NC6):
                    nc.sync.dma_start(
                        out=wv_sb[:, _c],
                        in_=wv[_c * 128:(_c + 1) * 128, :].bitcast(f32r))
                v = []
                for t in range(NT):
                    r = ROWS[t]
                    pV = ps.tile([128, 1024], f32, tag="ps")
                    for c in range(NC6):
                        nc.tensor.matmul(pV[:r, 0:512],
                                         hT[c][:, t * 128:t * 128 + r],
                                         wv_sb[:, c, 0:512],
                                         start=(c == 0), stop=(c == NC6 - 1))
                        nc.tensor.matmul(pV[:r, 512:768],
                                         hT[c][:, t * 128:t * 128 + r],
                                         wv_sb[:, c, 512:768],
                                         start=(c == 0), stop=(c == NC6 - 1))
                    vt = p768.tile([128, C], f32r, tag="a768")
                    if has_bv:
                        nc.vector.tensor_tensor(
                            out=vt[:r], in0=pV[:r, 0:768], in1=bv_bc[:r], op=ALU.add)
                    else:
                        nc.scalar.copy(vt[:r], pV[:r, 0:768])
                    v.append(vt)

                # ---------------- Phase C: S^T, exp, denom, r ----------------
                kTS = kT if has_qkb else hT
                PuT = []
                for m in range(NT):
                    rm = ROWS[m]
                    pS = ps.tile([128, 1024], f32, tag="ps")
                    for c in range(NC6):
                        for (so, ln, po) in CHUNKS:
                            nc.tensor.matmul(
                                pS[:rm, po:po + ln],
                                kTS[c][:, m * 128:m * 128 + rm],
                                qT[c][:, so:so + ln],
                                start=(c == 0), stop=(c == NC6 - 1))
                    pu = p577.tile([128, N_PAD], f32r, tag="a577")
                    for (so, ln, po) in CHUNKS:
                        nc.scalar.activation(
                            pu[:rm, so:so + ln], pS[:rm, po:po + ln],
                            AF.Exp, scale=scale)
                    PuT.append(pu)
                pD = ps.tile([128, 1024], f32, tag="ps")
                for m in range(NT):
                    rm = ROWS[m]
                    for (so, ln, po) in CHUNKS:
                        nc.tensor.matmul(
                            pD[0:1, po:po + ln], ones_col[:rm],
                            PuT[m][:rm, so:so + ln],
                            start=(m == 0), stop=(m == NT - 1))
                dsum = row577.tile([1, N_PAD], f32, tag="row")
                for (so, ln, po) in CHUNKS:
                    nc.scalar.copy(dsum[:, so:so + ln], pD[0:1, po:po + ln])
                rinv = row577.tile([1, N_PAD], f32, tag="row")
                nc.vector.reciprocal(rinv, dsum)
                pR = ps.tile([128, 1024], f32, tag="ps")
                for (so, ln, po) in CHUNKS:
                    nc.tensor.matmul(pR[:, po:po + ln], ones_row,
                                     rinv[:, so:so + ln], start=True, stop=True)
                rbc = p577.tile([128, N_PAD], f32, tag="a577")
                for (so, ln, po) in CHUNKS:
                    nc.scalar.copy(rbc[:, so:so + ln], pR[:, po:po + ln])

                # token_attn: normalized attention row 0 = PuT[m][:, 0] * r[0]
                ta = small.tile([128, NT], f32, tag="ta")
                for m in range(NT):
                    nc.vector.tensor_copy(ta[:ROWS[m], m:m + 1], PuT[m][:ROWS[m], 0:1])
                nc.vector.tensor_scalar_mul(ta, ta, rbc[:, 0:1])
                nc.sync.dma_start(
                    out=bass.AP(tensor=tattn_d[:].tensor, offset=b * (N - 1),
                                ap=[[1, 127]]),
                    in_=ta[1:128, 0:1])
                nc.sync.dma_start(
                    out=bass.AP(tensor=tattn_d[:].tensor, offset=b * (N - 1) + 127,
                                ap=[[1, 128], [128, 3]]),
                    in_=ta[:, 1:4])
                nc.sync.dma_start(
                    out=bass.AP(tensor=tattn_d[:].tensor, offset=b * (N - 1) + 511,
                                ap=[[1, 65]]),
                    in_=ta[0:65, 4:5])

                if b == 0:
                    nc.sync.dma_start(
                        out=pw2T_sb,
                        in_=pw2T[:, :].rearrange("(a p) o -> p a o", p=128))
                # ---------------- Phase D: y^T = (v^T @ Pu^T) * r ----------------
                ydram = dpool.tile([C * N], f32, tag="yd")
                yd_w = ydram.rearrange("(a b) -> a b", b=N)    # [C, N] write view
                yd_r = ydram.rearrange("(a b) -> a b", b=C)    # [N, C] read view
                for c in range(NC6):
                    pY = ps.tile([128, 1024], f32, tag="ps")
                    for m in range(NT):
                        rm = ROWS[m]
                        for (so, ln, po) in CHUNKS:
                            nc.tensor.matmul(
                                pY[:, po:po + ln],
                                v[m][:rm, c * 128:(c + 1) * 128],
                                PuT[m][:rm, so:so + ln],
                                start=(m == 0), stop=(m == NT - 1))
                    yc = p577.tile([128, N_PAD], f32, tag="a577")
                    for (so, ln, po) in CHUNKS:
                        nc.vector.tensor_tensor(
                            out=yc[:, so:so + ln], in0=pY[:, po:po + ln],
                            in1=rbc[:, so:so + ln], op=ALU.mult)
                    nc.sync.dma_start(out=yd_w[c * 128:(c + 1) * 128, :], in_=yc[:, :N])

                # ---------------- Phase E: scramble read, transpose, proj ----------------
                ys = []
                for t in range(NT):
                    r = ROWS[t]
                    yst = p768.tile([128, C], f32r, tag="a768")
                    nc.sync.dma_start(out=yst[:r],
                                      in_=yd_r[t * 128:t * 128 + r, :].bitcast(f32r))
                    ys.append(yst)
                ysT = []
                for c in range(NC6):
                    pT2 = ps.tile([128, 1024], f32r, tag="ps")
                    for t in range(NT):
                        r = ROWS[t] if ROWS[t] % 2 == 0 else ROWS[t] + 1
                        nc.tensor.transpose(
                            pT2[:, t * 128:t * 128 + r],
                            ys[t][:r, c * 128:(c + 1) * 128], ident_r[:r, :r])
                    yTc = p577.tile([128, N_PAD], f32r, tag="a577")
                    nc.scalar.copy(yTc, pT2[:, :N_PAD])
                    ysT.append(yTc)
                x2 = []
                for t in range(NT):
                    r = ROWS[t]
                    pP = ps.tile([128, 1024], f32, tag="ps")
                    for c in range(NC6):
                        nc.tensor.matmul(pP[:r, 0:512],
                                         ysT[c][:, t * 128:t * 128 + r],
                                         pw2T_sb[:, c, 0:512],
                                         start=(c == 0), stop=(c == NC6 - 1))
                        nc.tensor.matmul(pP[:r, 512:768],
                                         ysT[c][:, t * 128:t * 128 + r],
                                         pw2T_sb[:, c, 512:768],
                                         start=(c == 0), stop=(c == NC6 - 1))
                    x2t = p768.tile([128, C], f32, tag="a768")
                    nc.vector.tensor_tensor(
                        out=x2t[:r], in0=pP[:r, 0:768], in1=pb2_bc[:r], op=ALU.add)
                    x2.append(x2t)

                # ---------------- Phase F: LN2 + transpose -> mT ----------------
                mtk = []
                for t in range(NT):
                    r = ROWS[t]
                    st = small.tile([128, 2, 6], f32, tag="bn")
                    xg = x2[t].rearrange("p (g d) -> p g d", g=2)
                    nc.vector.bn_stats(out=st[:r, 0], in_=xg[:r, 0])
                    nc.vector.bn_stats(out=st[:r, 1], in_=xg[:r, 1])
                    mv = small.tile([128, 2], f32, tag="mv")
                    nc.vector.bn_aggr(out=mv[:r], in_=st[:r])
                    sd = small.tile([128, 1], f32, tag="sd")
                    nc.scalar.activation(sd[:r], mv[:r, 1:2], AF.Sqrt, bias=eps_t[:r])
                    rc = small.tile([128, 1], f32, tag="rc")
                    nc.vector.reciprocal(rc[:r], sd[:r])
                    nmr = small.tile([128, 1], f32, tag="nmr")
                    nc.vector.tensor_scalar(
                        nmr[:r], mv[:r, 0:1], rc[:r], -1.0, ALU.mult, ALU.mult)
                    mt = p768.tile([128, C], f32r, tag="a768")
                    nc.scalar.activation(mt[:r], x2[t][:r], AF.Identity,
                                         bias=nmr[:r], scale=rc[:r])
                    mtk.append(mt)
                mT = []
                for c in range(NC6):
                    pT3 = ps.tile([128, 1024], f32r, tag="ps")
                    for t in range(NT):
                        r = ROWS[t] if ROWS[t] % 2 == 0 else ROWS[t] + 1
                        nc.tensor.transpose(
                            pT3[:, t * 128:t * 128 + r],
                            mtk[t][:r, c * 128:(c + 1) * 128], ident_r[:r, :r])
                    mc = p577.tile([128, N_PAD], f32r, tag="a577")
                    nc.scalar.copy(mc, pT3[:, :N_PAD])
                    mT.append(mc)

                # ---------------- Phase G: fc1 (hf quarters) + fc2 ----------------
                if b + 1 < BPC:
                    xt_next = load_x(b + 1)
                g_sb = [None] * NT
                NGRP = 4
                GSZ = NH24 // NGRP
                for half in range(NGRP):
                    fT = []
                    for hl in range(GSZ):
                        hh = half * GSZ + hl
                        f1c = wchunk.tile([128, NC6, 128], f32r, tag="ws")
                        nc.sync.dma_start(
                            out=f1c,
                            in_=fc1wT[:, hh * 128:(hh + 1) * 128]
                            .rearrange("(a p) j -> p a j", p=128))
                        pF = ps.tile([128, 1024], f32, tag="ps")
                        for c in range(NC6):
                            for (so, ln, po) in CHUNKS:
                                nc.tensor.matmul(
                                    pF[:, po:po + ln], f1c[:, c, :],
                                    mT[c][:, so:so + ln],
                                    start=(c == 0), stop=(c == NC6 - 1))
                        ft = p577.tile([128, N_PAD], f32r, tag="a577")
                        for (so, ln, po) in CHUNKS:
                            nc.scalar.activation(
                                ft[:, so:so + ln], pF[:, po:po + ln],
                                AF.Gelu, bias=fc1b_sb[:, hh:hh + 1])
                        fT.append(ft)
                    f2c = []
                    for hl in range(GSZ):
                        hh = half * GSZ + hl
                        fc = fc2c_pool.tile([128, C], f32r, tag="f2c")
                        nc.sync.dma_start(out=fc, in_=fc2wT[hh * 128:(hh + 1) * 128, :])
                        f2c.append(fc)
                    if half == NGRP - 1:
                        wq_next = load_wq() if b + 1 < BPC else None
                    for t in range(NT):
                        r = ROWS[t]
                        pG = ps.tile([128, 1024], f32, tag="ps")
                        for hl in range(GSZ):
                            nc.tensor.matmul(pG[:r, 0:512],
                                             fT[hl][:, t * 128:t * 128 + r],
                                             f2c[hl][:, 0:512],
                                             start=(hl == 0), stop=(hl == GSZ - 1))
                            nc.tensor.matmul(pG[:r, 512:768],
                                             fT[hl][:, t * 128:t * 128 + r],
                                             f2c[hl][:, 512:768],
                                             start=(hl == 0), stop=(hl == GSZ - 1))
                        if half == 0:
                            gt = p768.tile([128, C], f32, tag="a768")
                            nc.vector.tensor_tensor(
                                out=gt[:r], in0=pG[:r, 0:768], in1=fc2b_bc[:r],
                                op=ALU.add)
                            g_sb[t] = gt
                        elif half < NGRP - 1:
                            nc.vector.tensor_tensor(
                                out=g_sb[t][:r], in0=pG[:r, 0:768],
                                in1=g_sb[t][:r], op=ALU.add)
                        else:
                            nc.vector.tensor_tensor(
                                out=g_sb[t][:r], in0=pG[:r, 0:768],
                                in1=g_sb[t][:r], op=ALU.add)
                            ot = p768.tile([128, C], f32, tag="a768")
                            nc.vector.tensor_tensor(
                                out=ot[:r], in0=g_sb[t][:r], in1=x2[t][:r],
                                op=ALU.add)
                            nc.sync.dma_start(
                                out=out_d[b, t * 128:t * 128 + r, :], in_=ot[:r])

    _legalize_sync(nc, mybir)
    return nc


_PROG_CACHE = {}


def _get_program(has_bv, has_qkb):
    key = (has_bv, has_qkb)
    if key not in _PROG_CACHE:
        _PROG_CACHE[key] = _build_program(has_bv, has_qkb)
    return _PROG_CACHE[key]


def _prep_host(x, norm1_w, norm1_b, qkv_w, proj_w, proj_b,
               norm2_w, norm2_b, fc1_w, fc1_b, fc2_w, fc2_b):
    f = np.float32
    x = np.ascontiguousarray(np.asarray(x, f))
    w1 = np.asarray(norm1_w, f); b1 = np.asarray(norm1_b, f)
    w2 = np.asarray(norm2_w, f); b2 = np.asarray(norm2_b, f)
    qkv_w = np.asarray(qkv_w, f); proj_w = np.asarray(proj_w, f)
    fc1_w = np.asarray(fc1_w, f); fc2_w = np.asarray(fc2_w, f)

    qkv_w_eff = qkv_w * w1[None, :]
    qkv_b_eff = qkv_w @ b1
    wqT = np.ascontiguousarray(qkv_w_eff[0:C].T)
    wkT = np.ascontiguousarray(qkv_w_eff[C:2 * C].T)
    wvT = np.ascontiguousarray(qkv_w_eff[2 * C:3 * C].T)
    bq = np.ascontiguousarray(qkv_b_eff[0:C].reshape(NC6, 128))
    bk = np.ascontiguousarray(qkv_b_eff[C:2 * C].reshape(NC6, 128))
    bv = np.ascontiguousarray(qkv_b_eff[2 * C:3 * C])
    pw2T = np.ascontiguousarray((2.0 * proj_w).T)
    pb2 = np.ascontiguousarray(2.0 * np.asarray(proj_b, f))
    fc1wT = np.ascontiguousarray((fc1_w * w2[None, :]).T)
    fc1b_eff = np.ascontiguousarray(
        (np.asarray(fc1_b, f) + fc1_w @ b2).reshape(NH24, 128))
    fc2wT = np.ascontiguousarray(fc2_w.T)
    fc2b = np.ascontiguousarray(np.asarray(fc2_b, f))
    return x, wqT, wkT, wvT, bq, bk, bv, pw2T, pb2, fc1wT, fc1b_eff, fc2wT, fc2b


def _run(inputs, trace=False):
    from concourse.bass_utils import run_bass_kernel_spmd

    (x, wqT, wkT, wvT, bq, bk, bv, pw2T, pb2, fc1wT, fc1b_eff,
     fc2wT, fc2b) = _prep_host(**inputs)
    has_bv = bool(np.any(bv))
    has_qkb = bool(np.any(bq)) or bool(np.any(bk))
    if not has_qkb:
        # S = q k^T = h (Wq^T Wk) h^T: stream A = Wq^T Wk instead of Wq, Wk
        # (wqT/wkT here are already transposed: wqT = Wq^T as [c, j]).
        A = (wqT.astype(np.float64) @ wkT.T.astype(np.float64)).astype(np.float32)
        wqT = np.ascontiguousarray(A)
    nc = _get_program(has_bv, has_qkb)

    in_maps = []
    for core in range(N_CORES):
        in_maps.append({
            "x": np.ascontiguousarray(x[core * BPC:(core + 1) * BPC]),
            "wq": wqT, "wk": wkT, "wv": wvT, "pw2T": pw2T,
            "fc1wT": fc1wT, "fc2wT": fc2wT,
            "bq2d": bq, "bk2d": bk, "fc1b2d": fc1b_eff,
            "pb2": pb2, "fc2b": fc2b, "bv1d": bv,
        })
    res = run_bass_kernel_spmd(nc, in_maps, core_ids=list(range(N_CORES)),
                               trace=trace)
    out = np.concatenate([res.results[i]["out"] for i in range(N_CORES)], axis=0)
    ta = np.concatenate([res.results[i]["tattn"] for i in range(N_CORES)], axis=0)
    asz = int((N - 1) ** 0.5)
    token_attn = ta.reshape(B, 1, asz, asz).astype(np.float32)
    return (out.astype(np.float32), token_attn), res


def kernel(**inputs):
    (out, token_attn), _ = _run(inputs, trace=False)
    return (out, token_attn)


# revision 38
# speedup vs baseline: 1.0126x; 1.0035x over previous
# Trainium2 Bass kernel for nn_Block (dense transformer block, single head).
#
# Strategy: pure data-parallel over batch. 32 batches / 8 cores = 4 per core.
# All weights replicated per core; no collectives.
#
# Math (per batch, faithful to reference):
#   h   = LN(x) * w1 + b1            (LN affine folded into qkv weights on host)
#   qkv = h @ qkv_w.T                (q,k channel-major; v token-major)
#   P   = softmax(q k^T / sqrt(C))   (computed as exp(S^T) + PE column-sum denom)
#   y   = P @ v                      (computed channel-major: y^T)
#   y_s = reshape(y^T, [N, C])       (torch transpose+reshape scramble; done via
#                                     a DRAM round-trip: write y^T flat, re-read
#                                     as [N, C] rows)
#   x2  = 2*(y_s @ proj_w.T + proj_b)  (factor 2 folded into proj weights)
#   out = x2 + fc2(gelu(fc1(LN(x2))))
#
# When norm1_b == 0 (always true for this problem's fixed init), q/k are never
# materialized: S = h (Wq^T Wk) h^T with A = Wq^T Wk precomputed on the host.
#
# Matmuls run as float32r (TF32-class mantissa, 4x the fp32 PE rate).

import numpy as np

N_CORES = 8
B = 32
BPC = B // N_CORES  # batches per core
N = 577
C = 768
H = 3072
EPS = 1e-5
NT = 5                     # token tiles of 128 (last has 65 rows)
ROWS = [128, 128, 128, 128, 65]
NC6 = C // 128             # 6 channel tiles
NH24 = H // 128            # 24 hidden tiles
# free-dim chunking of the 577-token axis: both >=256 so f32r runs 1 cyc/row.
# psum layout keeps chunk B in bank 1 (offset 512) so no matmul write
# straddles a 2KB psum bank boundary.
N_PAD = 578                # f32r matmuls need even free sizes; 577 is odd.
CHUNKS = [(0, 290, 0), (290, 288, 512)]   # (src_off, len, psum_off)


def _legalize_sync(nc, mybir):
    """Walrus allows only one sync-wait slot per hardware instruction (fp32/f32r
    matmuls are strictest); hoist excess waits onto InstNoOp carriers inserted
    immediately before, and excess updates onto following nops (never for DMA
    completion updates)."""
    n_fix = 0
    for f in nc.m.functions:
        for bb in f.blocks:
            il = bb.instructions
            out = []
            changed = False
            for inst in il:
                si = inst.sync_info
                waits = list(si.on_wait) if si is not None and si.on_wait else []
                upds = list(si.on_update) if si is not None and si.on_update else []
                if len(waits) > 1:
                    for w in waits[:-1]:
                        out.append(mybir.InstNoOp(
                            name=f"{inst.name}-wn{n_fix}-{len(out)}",
                            sync_info=mybir.SyncInfo(on_wait=[w], on_update=[]),
                            bass_nofuse=True,
                            engine=inst.engine,
                        ))
                    inst.sync_info = mybir.SyncInfo(on_wait=waits[-1:], on_update=upds)
                    changed = True
                    n_fix += 1
                out.append(inst)
                is_dma = isinstance(inst, (mybir.InstDMACopy, mybir.InstDMA,
                                           mybir.InstLoad, mybir.InstSave,
                                           mybir.InstTensorLoad, mybir.InstTensorSave))
                if not is_dma and len(upds) > 2:
                    si2 = inst.sync_info
                    inst.sync_info = mybir.SyncInfo(
                        on_wait=list(si2.on_wait), on_update=upds[:2])
                    for u in upds[2:]:
                        out.append(mybir.InstNoOp(
                            name=f"{inst.name}-un{n_fix}-{len(out)}",
                            sync_info=mybir.SyncInfo(on_wait=[], on_update=[u]),
                            bass_nofuse=True,
                            engine=inst.engine,
                        ))
                    changed = True
                    n_fix += 1
            if changed:
                bb.instructions = out
    return n_fix


def _build_program(has_bv, has_qkb):
    import concourse.bass as bass
    import concourse.mybir as mybir
    from concourse.tile import TileContext
    from concourse.masks import make_identity

    f32 = mybir.dt.float32
    f32r = mybir.dt.float32r
    AF = mybir.ActivationFunctionType
    ALU = mybir.AluOpType

    nc = bass.Bass()

    # --- DRAM parameters (per core) ---
    x_in = nc.declare_dram_parameter("x", [BPC, N, C], f32, isOutput=False)
    wq = nc.declare_dram_parameter("wq", [C, C], f32r, isOutput=False)
    wk = nc.declare_dram_parameter("wk", [C, C], f32r, isOutput=False)
    wv = nc.declare_dram_parameter("wv", [C, C], f32r, isOutput=False)
    pw2T = nc.declare_dram_parameter("pw2T", [C, C], f32r, isOutput=False)
    fc1wT = nc.declare_dram_parameter("fc1wT", [C, H], f32r, isOutput=False)
    fc2wT = nc.declare_dram_parameter("fc2wT", [H, C], f32r, isOutput=False)
    bq2d = nc.declare_dram_parameter("bq2d", [NC6, 128], f32, isOutput=False)
    bk2d = nc.declare_dram_parameter("bk2d", [NC6, 128], f32, isOutput=False)
    fc1b2d = nc.declare_dram_parameter("fc1b2d", [NH24, 128], f32, isOutput=False)
    pb2 = nc.declare_dram_parameter("pb2", [C], f32, isOutput=False)
    fc2b = nc.declare_dram_parameter("fc2b", [C], f32, isOutput=False)
    bv1d = nc.declare_dram_parameter("bv1d", [C], f32, isOutput=False)
    out_d = nc.declare_dram_parameter("out", [BPC, N, C], f32, isOutput=True)
    tattn_d = nc.declare_dram_parameter("tattn", [BPC, N - 1], f32, isOutput=True)

    scale = float(C) ** -0.5

    def bcast_ap(param, n):
        ap = param[:]
        return bass.AP(tensor=ap.tensor, offset=0, ap=[[0, 128], [1, n]])

    def t_ap(param, rows, cols):
        # [rows, cols] dram -> [cols(part), rows(free)] transposed load AP
        ap = param[:]
        return bass.AP(tensor=ap.tensor, offset=0, ap=[[1, cols], [cols, rows]])

    with TileContext(nc) as tc:
        with (
            tc.tile_pool(name="consts", bufs=1) as consts,
            tc.tile_pool(name="small", bufs=10) as small,
            tc.tile_pool(name="row577", bufs=2) as row577,
            tc.tile_pool(name="p577", bufs=20) as p577,
            tc.tile_pool(name="p768", bufs=12) as p768,
            tc.tile_pool(name="xp", bufs=8) as xp,
            tc.tile_pool(name="wchunk", bufs=6) as wchunk,
            tc.tile_pool(name="fc2c", bufs=9) as fc2c_pool,
            tc.tile_pool(name="wvp", bufs=1) as wvp,
            tc.tile_pool(name="ps", bufs=4, space="PSUM") as ps,
            tc.tile_pool(name="dram", bufs=2, space="DRAM") as dpool,
        ):
            # --- constants ---
            ident = consts.tile([128, 128], f32)
            make_identity(nc, ident)
            ident_r = consts.tile([128, 128], f32r)
            nc.scalar.copy(ident_r, ident)
            ones_f = consts.tile([128, 1], f32)
            nc.vector.memset(ones_f, 1.0)
            ones_col = consts.tile([128, 1], f32r)
            nc.scalar.copy(ones_col, ones_f)
            ones_row = consts.tile([1, 128], f32)
            nc.vector.memset(ones_row, 1.0)
            eps_t = consts.tile([128, 1], f32)
            nc.vector.memset(eps_t, EPS)
            bq_sb = consts.tile([128, NC6], f32)
            nc.sync.dma_start(out=bq_sb, in_=t_ap(bq2d, NC6, 128))
            bk_sb = consts.tile([128, NC6], f32)
            nc.sync.dma_start(out=bk_sb, in_=t_ap(bk2d, NC6, 128))
            fc1b_sb = consts.tile([128, NH24], f32)
            nc.sync.dma_start(out=fc1b_sb, in_=t_ap(fc1b2d, NH24, 128))
            pb2_bc = consts.tile([128, C], f32)
            nc.sync.dma_start(out=pb2_bc, in_=bcast_ap(pb2, C))
            fc2b_bc = consts.tile([128, C], f32)
            nc.sync.dma_start(out=fc2b_bc, in_=bcast_ap(fc2b, C))
            if has_bv:
                bv_bc = consts.tile([128, C], f32)
                nc.sync.dma_start(out=bv_bc, in_=bcast_ap(bv1d, C))
            pw2T_sb = consts.tile([128, NC6, C], f32r)

            def load_x(b):
                tiles = []
                for t in range(NT):
                    r = ROWS[t]
                    xt = xp.tile([128, C], f32, tag="xt")
                    nc.gpsimd.dma_start(out=xt[:r], in_=x_in[b, t * 128:t * 128 + r, :])
                    tiles.append(xt)
                return tiles

            def load_wq():
                tiles = []
                for j in range(NC6):
                    wc = wchunk.tile([128, NC6, 128], f32r, tag="ws")
                    nc.sync.dma_start(
                        out=wc,
                        in_=wq[:, j * 128:(j + 1) * 128]
                        .rearrange("(a p) j -> p a j", p=128))
                    tiles.append(wc)
                return tiles

            def compute_ht(xts):
                tiles = []
                for t in range(NT):
                    r = ROWS[t]
                    xt = xts[t]
                    st = small.tile([128, 2, 6], f32, tag="bn")
                    xg = xt.rearrange("p (g d) -> p g d", g=2)
                    nc.vector.bn_stats(out=st[:r, 0], in_=xg[:r, 0])
                    nc.vector.bn_stats(out=st[:r, 1], in_=xg[:r, 1])
                    mv = small.tile([128, 2], f32, tag="mv")
                    nc.vector.bn_aggr(out=mv[:r], in_=st[:r])
                    sd = small.tile([128, 1], f32, tag="sd")
                    nc.scalar.activation(sd[:r], mv[:r, 1:2], AF.Sqrt, bias=eps_t[:r])
                    rc = small.tile([128, 1], f32, tag="rc")
                    nc.vector.reciprocal(rc[:r], sd[:r])
                    nmr = small.tile([128, 1], f32, tag="nmr")
                    nc.vector.tensor_scalar(
                        nmr[:r], mv[:r, 0:1], rc[:r], -1.0, ALU.mult, ALU.mult)
                    h = xp.tile([128, C], f32r, tag="xt")
                    nc.scalar.activation(h[:r], xt[:r], AF.Identity,
                                         bias=nmr[:r], scale=rc[:r])
                    tiles.append(h)
                return tiles

            xt_next = load_x(0)
            ht_next = compute_ht(xt_next)
            wq_next = None
            for b in range(BPC):
                # ---------------- Phase A: transpose ht -> hT ----------------
                ht = ht_next
                hT = []
                for c in range(NC6):
                    pA = ps.tile([128, 1024], f32r, tag="ps")
                    for t in range(NT):
                        r = ROWS[t] if ROWS[t] % 2 == 0 else ROWS[t] + 1
                        nc.tensor.transpose(
                            pA[:, t * 128:t * 128 + r],
                            ht[t][:r, c * 128:(c + 1) * 128], ident_r[:r, :r])
                    hc = p577.tile([128, N_PAD], f32r, tag="a577")
                    nc.scalar.copy(hc, pA[:, :N_PAD])
                    hT.append(hc)

                # ---------------- Phase B: qkv ----------------
                qT, kT = [], []
                wsrcs = ((wq, bq_sb, qT), (wk, bk_sb, kT)) if has_qkb \
                    else ((wq, bq_sb, qT),)
                for w_par, b_sb, dst in wsrcs:
                    for j in range(NC6):
                        if w_par is wq and wq_next is not None:
                            wc = wq_next[j]
                        else:
                            wc = wchunk.tile([128, NC6, 128], f32r, tag="ws")
                            nc.sync.dma_start(
                                out=wc,
                                in_=w_par[:, j * 128:(j + 1) * 128]
                                .rearrange("(a p) j -> p a j", p=128))
                        pQ = ps.tile([128, 1024], f32, tag="ps")
                        for c in range(NC6):
                            for (so, ln, po) in CHUNKS:
                                nc.tensor.matmul(
                                    pQ[:, po:po + ln], wc[:, c, :],
                                    hT[c][:, so:so + ln],
                                    start=(c == 0), stop=(c == NC6 - 1))
                        qj = p577.tile([128, N_PAD], f32r, tag="a577")
                        for (so, ln, po) in CHUNKS:
                            nc.scalar.activation(
                                qj[:, so:so + ln], pQ[:, po:po + ln],
                                AF.Identity, bias=b_sb[:, j:j + 1])
                        dst.append(qj)
                wv_sb = wvp.tile([128, NC6, C], f32r, tag="wv")
                for _c in range(NC6):
                    nc.sync.dma_start(
                        out=wv_sb[:, _c],
                        in_=wv[_c * 128:(_c + 1) * 128, :].bitcast(f32r))
                v = []
                for t in range(NT):
                    r = ROWS[t]
                    pV = ps.tile([128, 1024], f32, tag="ps")
                    for c in range(NC6):
                        nc.tensor.matmul(pV[:r, 0:512],
                                         hT[c][:, t * 128:t * 128 + r],
                                         wv_sb[:, c, 0:512],
                                         start=(c == 0), stop=(c == NC6 - 1))
                        nc.tensor.matmul(pV[:r, 512:768],
                                         hT[c][:, t * 128:t * 128 + r],
                                         wv_sb[:, c, 512:768],
                                         start=(c == 0), stop=(c == NC6 - 1))
                    vt = p768.tile([128, C], f32r, tag="a768")
                    if has_bv:
                        nc.vector.tensor_tensor(
                            out=vt[:r], in0=pV[:r, 0:768], in1=bv_bc[:r], op=ALU.add)
                    else:
                        nc.scalar.copy(vt[:r], pV[:r, 0:768])
                    v.append(vt)

                # ---------------- Phase C: S^T, exp, denom, r ----------------
                kTS = kT if has_qkb else hT
                PuT = []
                for m in range(NT):
                    rm = ROWS[m]
                    pS = ps.tile([128, 1024], f32, tag="ps")
                    for c in range(NC6):
                        for (so, ln, po) in CHUNKS:
                            nc.tensor.matmul(
                                pS[:rm, po:po + ln],
                                kTS[c][:, m * 128:m * 128 + rm],
                                qT[c][:, so:so + ln],
                                start=(c == 0), stop=(c == NC6 - 1))
                    pu = p577.tile([128, N_PAD], f32r, tag="a577")
                    for (so, ln, po) in CHUNKS:
                        nc.scalar.activation(
                            pu[:rm, so:so + ln], pS[:rm, po:po + ln],
                            AF.Exp, scale=scale)
                    PuT.append(pu)
                pD = ps.tile([128, 1024], f32, tag="ps")
                for m in range(NT):
                    rm = ROWS[m]
                    for (so, ln, po) in CHUNKS:
                        nc.tensor.matmul(
                            pD[0:1, po:po + ln], ones_col[:rm],
                            PuT[m][:rm, so:so + ln],
                            start=(m == 0), stop=(m == NT - 1))
                dsum = row577.tile([1, N_PAD], f32, tag="row")
                for (so, ln, po) in CHUNKS:
                    nc.scalar.copy(dsum[:, so:so + ln], pD[0:1, po:po + ln])
                rinv = row577.tile([1, N_PAD], f32, tag="row")
                nc.vector.reciprocal(rinv, dsum)
                pR = ps.tile([128, 1024], f32, tag="ps")
                for (so, ln, po) in CHUNKS:
                    nc.tensor.matmul(pR[:, po:po + ln], ones_row,
                                     rinv[:, so:so + ln], start=True, stop=True)
                rbc = p577.tile([128, N_PAD], f32, tag="a577")
                for (so, ln, po) in CHUNKS:
                    nc.scalar.copy(rbc[:, so:so + ln], pR[:, po:po + ln])

                # token_attn: normalized attention row 0 = PuT[m][:, 0] * r[0]
                ta = small.tile([128, NT], f32, tag="ta")
                for m in range(NT):
                    nc.vector.tensor_copy(ta[:ROWS[m], m:m + 1], PuT[m][:ROWS[m], 0:1])
                nc.vector.tensor_scalar_mul(ta, ta, rbc[:, 0:1])
                nc.sync.dma_start(
                    out=bass.AP(tensor=tattn_d[:].tensor, offset=b * (N - 1),
                                ap=[[1, 127]]),
                    in_=ta[1:128, 0:1])
                nc.sync.dma_start(
                    out=bass.AP(tensor=tattn_d[:].tensor, offset=b * (N - 1) + 127,
                                ap=[[1, 128], [128, 3]]),
                    in_=ta[:, 1:4])
                nc.sync.dma_start(
                    out=bass.AP(tensor=tattn_d[:].tensor, offset=b * (N - 1) + 511,
                                ap=[[1, 65]]),
                    in_=ta[0:65, 4:5])

                if b == 0:
                    nc.sync.dma_start(
                        out=pw2T_sb,
                        in_=pw2T[:, :].rearrange("(a p) o -> p a o", p=128))
                # ---------------- Phase D: y^T = (v^T @ Pu^T) * r ----------------
                ydram = dpool.tile([C * N], f32, tag="yd")
                yd_w = ydram.rearrange("(a b) -> a b", b=N)    # [C, N] write view
                yd_r = ydram.rearrange("(a b) -> a b", b=C)    # [N, C] read view
                for c in range(NC6):
                    pY = ps.tile([128, 1024], f32, tag="ps")
                    for m in range(NT):
                        rm = ROWS[m]
                        for (so, ln, po) in CHUNKS:
                            nc.tensor.matmul(
                                pY[:, po:po + ln],
                                v[m][:rm, c * 128:(c + 1) * 128],
                                PuT[m][:rm, so:so + ln],
                                start=(m == 0), stop=(m == NT - 1))
                    yc = p577.tile([128, N_PAD], f32, tag="a577")
                    for (so, ln, po) in CHUNKS:
                        nc.vector.tensor_tensor(
                            out=yc[:, so:so + ln], in0=pY[:, po:po + ln],
                            in1=rbc[:, so:so + ln], op=ALU.mult)
                    nc.sync.dma_start(out=yd_w[c * 128:(c + 1) * 128, :], in_=yc[:, :N])

                # ---------------- Phase E: scramble read, transpose, proj ----------------
                ys = []
                for t in range(NT):
                    r = ROWS[t]
                    yst = p768.tile([128, C], f32r, tag="a768")
                    nc.sync.dma_start(out=yst[:r],
                                      in_=yd_r[t * 128:t * 128 + r, :].bitcast(f32r))
                    ys.append(yst)
                ysT = []
                for c in range(NC6):
                    pT2 = ps.tile([128, 1024], f32r, tag="ps")
                    for t in range(NT):
                        r = ROWS[t] if ROWS[t] % 2 == 0 else ROWS[t] + 1
                        nc.tensor.transpose(
                            pT2[:, t * 128:t * 128 + r],
                            ys[t][:r, c * 128:(c + 1) * 128], ident_r[:r, :r])
                    yTc = p577.tile([128, N_PAD], f32r, tag="a577")
                    nc.scalar.copy(yTc, pT2[:, :N_PAD])
                    ysT.append(yTc)
                x2 = []
                for t in range(NT):
                    r = ROWS[t]
                    pP = ps.tile([128, 1024], f32, tag="ps")
                    for c in range(NC6):
                        nc.tensor.matmul(pP[:r, 0:512],
                                         ysT[c][:, t * 128:t * 128 + r],
                                         pw2T_sb[:, c, 0:512],
                                         start=(c == 0), stop=(c == NC6 - 1))
                        nc.tensor.matmul(pP[:r, 512:768],
                                         ysT[c][:, t * 128:t * 128 + r],
                                         pw2T_sb[:, c, 512:768],
                                         start=(c == 0), stop=(c == NC6 - 1))
                    x2t = p768.tile([128, C], f32, tag="a768")
                    nc.vector.tensor_tensor(
                        out=x2t[:r], in0=pP[:r, 0:768], in1=pb2_bc[:r], op=ALU.add)
                    x2.append(x2t)

                # ---------------- Phase F: LN2 + transpose -> mT ----------------
                mtk = []
                for t in range(NT):
                    r = ROWS[t]
                    st = small.tile([128, 2, 6], f32, tag="bn")
                    xg = x2[t].rearrange("p (g d) -> p g d", g=2)
                    nc.vector.bn_stats(out=st[:r, 0], in_=xg[:r, 0])
                    nc.vector.bn_stats(out=st[:r, 1], in_=xg[:r, 1])
                    mv = small.tile([128, 2], f32, tag="mv")
                    nc.vector.bn_aggr(out=mv[:r], in_=st[:r])
                    sd = small.tile([128, 1], f32, tag="sd")
                    nc.scalar.activation(sd[:r], mv[:r, 1:2], AF.Sqrt, bias=eps_t[:r])
                    rc = small.tile([128, 1], f32, tag="rc")
                    nc.vector.reciprocal(rc[:r], sd[:r])
                    nmr = small.tile([128, 1], f32, tag="nmr")
                    nc.vector.tensor_scalar(
                        nmr[:r], mv[:r, 0:1], rc[:r], -1.0, ALU.mult, ALU.mult)
                    mt = p768.tile([128, C], f32r, tag="a768")
                    nc.scalar.activation(mt[:r], x2[t][:r], AF.Identity,
                                         bias=nmr[:r], scale=rc[:r])
                    mtk.append(mt)
                mT = []
                for c in range(NC6):
                    pT3 = ps.tile([128, 1024], f32r, tag="ps")
                    for t in range(NT):
                        r = ROWS[t] if ROWS[t] % 2 == 0 else ROWS[t] + 1
                        nc.tensor.transpose(
                            pT3[:, t * 128:t * 128 + r],
                            mtk[t][:r, c * 128:(c + 1) * 128], ident_r[:r, :r])
                    mc = p577.tile([128, N_PAD], f32r, tag="a577")
                    nc.scalar.copy(mc, pT3[:, :N_PAD])
                    mT.append(mc)

                # ---------------- Phase G: fc1 chunks + fc2 ----------------
                if b + 1 < BPC:
                    xt_next = load_x(b + 1)
                    ht_next = compute_ht(xt_next)
                g_sb = [None] * NT
                NGRP = 3
                GSZ = NH24 // NGRP
                for half in range(NGRP):
                    fT = []
                    for hl in range(GSZ):
                        hh = half * GSZ + hl
                        f1c = wchunk.tile([128, NC6, 128], f32r, tag="ws")
                        nc.sync.dma_start(
                            out=f1c,
                            in_=fc1wT[:, hh * 128:(hh + 1) * 128]
                            .rearrange("(a p) j -> p a j", p=128))
                        pF = ps.tile([128, 1024], f32, tag="ps")
                        for c in range(NC6):
                            for (so, ln, po) in CHUNKS:
                                nc.tensor.matmul(
                                    pF[:, po:po + ln], f1c[:, c, :],
                                    mT[c][:, so:so + ln],
                                    start=(c == 0), stop=(c == NC6 - 1))
                        ft = p577.tile([128, N_PAD], f32r, tag="a577")
                        for (so, ln, po) in CHUNKS:
                            nc.scalar.activation(
                                ft[:, so:so + ln], pF[:, po:po + ln],
                                AF.Gelu, bias=fc1b_sb[:, hh:hh + 1])
                        fT.append(ft)
                    f2c = []
                    for hl in range(GSZ):
                        hh = half * GSZ + hl
                        fc = fc2c_pool.tile([128, C], f32r, tag="f2c")
                        nc.sync.dma_start(out=fc, in_=fc2wT[hh * 128:(hh + 1) * 128, :])
                        f2c.append(fc)
                    if half == NGRP - 1:
                        wq_next = load_wq() if b + 1 < BPC else None
                    for t in range(NT):
                        r = ROWS[t]
                        pG = ps.tile([128, 1024], f32, tag="ps")
                        for hl in range(GSZ):
                            nc.tensor.matmul(pG[:r, 0:512],
                                             fT[hl][:, t * 128:t * 128 + r],
                                             f2c[hl][:, 0:512],
                                             start=(hl == 0), stop=(hl == GSZ - 1))
                            nc.tensor.matmul(pG[:r, 512:768],
                                             fT[hl][:, t * 128:t * 128 + r],
                                             f2c[hl][:, 512:768],
                                             start=(hl == 0), stop=(hl == GSZ - 1))
                        if half == 0:
                            gt = p768.tile([128, C], f32, tag="a768")
                            nc.vector.tensor_tensor(
                                out=gt[:r], in0=pG[:r, 0:768], in1=fc2b_bc[:r],
                                op=ALU.add)
                            g_sb[t] = gt
                        elif half < NGRP - 1:
                            nc.vector.tensor_tensor(
                                out=g_sb[t][:r], in0=pG[:r, 0:768],
                                in1=g_sb[t][:r], op=ALU.add)
                        else:
                            nc.vector.tensor_tensor(
                                out=g_sb[t][:r], in0=pG[:r, 0:768],
                                in1=g_sb[t][:r], op=ALU.add)
                            ot = p768.tile([128, C], f32, tag="a768")
                            nc.vector.tensor_tensor(
                                out=ot[:r], in0=g_sb[t][:r], in1=x2[t][:r],
                                op=ALU.add)
                            nc.sync.dma_start(
                                out=out_d[b, t * 128:t * 128 + r, :], in_=ot[:r])

    _legalize_sync(nc, mybir)
    return nc


_PROG_CACHE = {}


def _get_program(has_bv, has_qkb):
    key = (has_bv, has_qkb)
    if key not in _PROG_CACHE:
        _PROG_CACHE[key] = _build_program(has_bv, has_qkb)
    return _PROG_CACHE[key]


def _prep_host(x, norm1_w, norm1_b, qkv_w, proj_w, proj_b,
               norm2_w, norm2_b, fc1_w, fc1_b, fc2_w, fc2_b):
    f = np.float32
    x = np.ascontiguousarray(np.asarray(x, f))
    w1 = np.asarray(norm1_w, f); b1 = np.asarray(norm1_b, f)
    w2 = np.asarray(norm2_w, f); b2 = np.asarray(norm2_b, f)
    qkv_w = np.asarray(qkv_w, f); proj_w = np.asarray(proj_w, f)
    fc1_w = np.asarray(fc1_w, f); fc2_w = np.asarray(fc2_w, f)

    qkv_w_eff = qkv_w * w1[None, :]
    qkv_b_eff = qkv_w @ b1
    wqT = np.ascontiguousarray(qkv_w_eff[0:C].T)
    wkT = np.ascontiguousarray(qkv_w_eff[C:2 * C].T)
    wvT = np.ascontiguousarray(qkv_w_eff[2 * C:3 * C].T)
    bq = np.ascontiguousarray(qkv_b_eff[0:C].reshape(NC6, 128))
    bk = np.ascontiguousarray(qkv_b_eff[C:2 * C].reshape(NC6, 128))
    bv = np.ascontiguousarray(qkv_b_eff[2 * C:3 * C])
    pw2T = np.ascontiguousarray((2.0 * proj_w).T)
    pb2 = np.ascontiguousarray(2.0 * np.asarray(proj_b, f))
    fc1wT = np.ascontiguousarray((fc1_w * w2[None, :]).T)
    fc1b_eff = np.ascontiguousarray(
        (np.asarray(fc1_b, f) + fc1_w @ b2).reshape(NH24, 128))
    fc2wT = np.ascontiguousarray(fc2_w.T)
    fc2b = np.ascontiguousarray(np.asarray(fc2_b, f))
    return x, wqT, wkT, wvT, bq, bk, bv, pw2T, pb2, fc1wT, fc1b_eff, fc2wT, fc2b


def _run(inputs, trace=False):
    from concourse.bass_utils import run_bass_kernel_spmd

    (x, wqT, wkT, wvT, bq, bk, bv, pw2T, pb2, fc1wT, fc1b_eff,
     fc2wT, fc2b) = _prep_host(**inputs)
    has_bv = bool(np.any(bv))
    has_qkb = bool(np.any(bq)) or bool(np.any(bk))
    if not has_qkb:
        # S = q k^T = h (Wq^T Wk) h^T: stream A = Wq^T Wk instead of Wq, Wk
        # (wqT/wkT here are already transposed: wqT = Wq^T as [c, j]).
        A = (wqT.astype(np.float64) @ wkT.T.astype(np.float64)).astype(np.float32)
        wqT = np.ascontiguousarray(A)
    nc = _get_program(has_bv, has_qkb)

    in_maps = []
    for core in range(N_CORES):
        in_maps.append({
            "x": np.ascontiguousarray(x[core * BPC:(core + 1) * BPC]),
            "wq": wqT, "wk": wkT, "wv": wvT, "pw2T": pw2T,
            "fc1wT": fc1wT, "fc2wT": fc2wT,
            "bq2d": bq, "bk2d": bk, "fc1b2d": fc1b_eff,
            "pb2": pb2, "fc2b": fc2b, "bv1d": bv,
        })
    res = run_bass_kernel_spmd(nc, in_maps, core_ids=list(range(N_CORES)),
                               trace=trace)
    out = np.concatenate([res.results[i]["out"] for i in range(N_CORES)], axis=0)
    ta = np.concatenate([res.results[i]["tattn"] for i in range(N_CORES)], axis=0)
    asz = int((N - 1) ** 0.5)
    token_attn = ta.reshape(B, 1, asz, asz).astype(np.float32)
    return (out.astype(np.float32), token_attn), res


def kernel(**inputs):
    (out, token_attn), _ = _run(inputs, trace=False)
    return (out, token_attn)


# revision 42
# speedup vs baseline: 1.0193x; 1.0066x over previous
# Trainium2 Bass kernel for nn_Block (dense transformer block, single head).
#
# Strategy: pure data-parallel over batch. 32 batches / 8 cores = 4 per core.
# All weights replicated per core; no collectives.
#
# Math (per batch, faithful to reference):
#   h   = LN(x) * w1 + b1            (LN affine folded into qkv weights on host)
#   qkv = h @ qkv_w.T                (q,k channel-major; v token-major)
#   P   = softmax(q k^T / sqrt(C))   (computed as exp(S^T) + PE column-sum denom)
#   y   = P @ v                      (computed channel-major: y^T)
#   y_s = reshape(y^T, [N, C])       (torch transpose+reshape scramble; done via
#                                     a DRAM round-trip: write y^T flat, re-read
#                                     as [N, C] rows)
#   x2  = 2*(y_s @ proj_w.T + proj_b)  (factor 2 folded into proj weights)
#   out = x2 + fc2(gelu(fc1(LN(x2))))
#
# When norm1_b == 0 (always true for this problem's fixed init), q/k are never
# materialized: S = h (Wq^T Wk) h^T with A = Wq^T Wk precomputed on the host.
#
# Matmuls run as float32r (TF32-class mantissa, 4x the fp32 PE rate).

import numpy as np

N_CORES = 8
B = 32
BPC = B // N_CORES  # batches per core
N = 577
C = 768
H = 3072
EPS = 1e-5
NT = 5                     # token tiles of 128 (last has 65 rows)
ROWS = [128, 128, 128, 128, 65]
NC6 = C // 128             # 6 channel tiles
NH24 = H // 128            # 24 hidden tiles
# free-dim chunking of the 577-token axis: both >=256 so f32r runs 1 cyc/row.
# psum layout keeps chunk B in bank 1 (offset 512) so no matmul write
# straddles a 2KB psum bank boundary.
N_PAD = 578                # f32r matmuls need even free sizes; 577 is odd.
CHUNKS = [(0, 290, 0), (290, 288, 512)]   # (src_off, len, psum_off)


def _legalize_sync(nc, mybir):
    """Walrus allows only one sync-wait slot per hardware instruction (fp32/f32r
    matmuls are strictest); hoist excess waits onto InstNoOp carriers inserted
    immediately before, and excess updates onto following nops (never for DMA
    completion updates)."""
    n_fix = 0
    for f in nc.m.functions:
        for bb in f.blocks:
            il = bb.instructions
            out = []
            changed = False
            for inst in il:
                si = inst.sync_info
                waits = list(si.on_wait) if si is not None and si.on_wait else []
                upds = list(si.on_update) if si is not None and si.on_update else []
                if len(waits) > 1:
                    for w in waits[:-1]:
                        out.append(mybir.InstNoOp(
                            name=f"{inst.name}-wn{n_fix}-{len(out)}",
                            sync_info=mybir.SyncInfo(on_wait=[w], on_update=[]),
                            bass_nofuse=True,
                            engine=inst.engine,
                        ))
                    inst.sync_info = mybir.SyncInfo(on_wait=waits[-1:], on_update=upds)
                    changed = True
                    n_fix += 1
                out.append(inst)
                is_dma = isinstance(inst, (mybir.InstDMACopy, mybir.InstDMA,
                                           mybir.InstLoad, mybir.InstSave,
                                           mybir.InstTensorLoad, mybir.InstTensorSave))
                if not is_dma and len(upds) > 2:
                    si2 = inst.sync_info
                    inst.sync_info = mybir.SyncInfo(
                        on_wait=list(si2.on_wait), on_update=upds[:2])
                    for u in upds[2:]:
                        out.append(mybir.InstNoOp(
                            name=f"{inst.name}-un{n_fix}-{len(out)}",
                            sync_info=mybir.SyncInfo(on_wait=[], on_update=[u]),
                            bass_nofuse=True,
                            engine=inst.engine,
                        ))
                    changed = True
                    n_fix += 1
            if changed:
                bb.instructions = out
    return n_fix


def _build_program(has_bv, has_qkb):
    import concourse.bass as bass
    import concourse.mybir as mybir
    from concourse.tile import TileContext
    from concourse.masks import make_identity

    f32 = mybir.dt.float32
    f32r = mybir.dt.float32r
    AF = mybir.ActivationFunctionType
    ALU = mybir.AluOpType

    nc = bass.Bass()

    # --- DRAM parameters (per core) ---
    x_in = nc.declare_dram_parameter("x", [BPC, N, C], f32, isOutput=False)
    wq = nc.declare_dram_parameter("wq", [C, C], f32r, isOutput=False)
    wk = nc.declare_dram_parameter("wk", [C, C], f32r, isOutput=False)
    wv = nc.declare_dram_parameter("wv", [C, C], f32r, isOutput=False)
    pw2T = nc.declare_dram_parameter("pw2T", [C, C], f32r, isOutput=False)
    fc1wT = nc.declare_dram_parameter("fc1wT", [C, H], f32r, isOutput=False)
    fc2wT = nc.declare_dram_parameter("fc2wT", [H, C], f32r, isOutput=False)
    bq2d = nc.declare_dram_parameter("bq2d", [NC6, 128], f32, isOutput=False)
    bk2d = nc.declare_dram_parameter("bk2d", [NC6, 128], f32, isOutput=False)
    fc1b2d = nc.declare_dram_parameter("fc1b2d", [NH24, 128], f32, isOutput=False)
    pb2 = nc.declare_dram_parameter("pb2", [C], f32, isOutput=False)
    fc2b = nc.declare_dram_parameter("fc2b", [C], f32, isOutput=False)
    bv1d = nc.declare_dram_parameter("bv1d", [C], f32, isOutput=False)
    out_d = nc.declare_dram_parameter("out", [BPC, N, C], f32, isOutput=True)
    tattn_d = nc.declare_dram_parameter("tattn", [BPC, N - 1], f32, isOutput=True)

    scale = float(C) ** -0.5

    def bcast_ap(param, n):
        ap = param[:]
        return bass.AP(tensor=ap.tensor, offset=0, ap=[[0, 128], [1, n]])

    def t_ap(param, rows, cols):
        # [rows, cols] dram -> [cols(part), rows(free)] transposed load AP
        ap = param[:]
        return bass.AP(tensor=ap.tensor, offset=0, ap=[[1, cols], [cols, rows]])

    with TileContext(nc) as tc:
        with (
            tc.tile_pool(name="consts", bufs=1) as consts,
            tc.tile_pool(name="small", bufs=10) as small,
            tc.tile_pool(name="row577", bufs=2) as row577,
            tc.tile_pool(name="p577", bufs=20) as p577,
            tc.tile_pool(name="p768", bufs=12) as p768,
            tc.tile_pool(name="xp", bufs=8) as xp,
            tc.tile_pool(name="wchunk", bufs=6) as wchunk,
            tc.tile_pool(name="fc2c", bufs=9) as fc2c_pool,
            tc.tile_pool(name="wvp", bufs=1) as wvp,
            tc.tile_pool(name="ps", bufs=4, space="PSUM") as ps,
            tc.tile_pool(name="dram", bufs=2, space="DRAM") as dpool,
        ):
            # --- constants ---
            ident = consts.tile([128, 128], f32)
            make_identity(nc, ident)
            ident_r = consts.tile([128, 128], f32r)
            nc.scalar.copy(ident_r, ident)
            ones_f = consts.tile([128, 1], f32)
            nc.vector.memset(ones_f, 1.0)
            ones_col = consts.tile([128, 1], f32r)
            nc.scalar.copy(ones_col, ones_f)
            ones_row = consts.tile([1, 128], f32)
            nc.vector.memset(ones_row, 1.0)
            eps_t = consts.tile([128, 1], f32)
            nc.vector.memset(eps_t, EPS)
            bq_sb = consts.tile([128, NC6], f32)
            nc.sync.dma_start(out=bq_sb, in_=t_ap(bq2d, NC6, 128))
            bk_sb = consts.tile([128, NC6], f32)
            nc.sync.dma_start(out=bk_sb, in_=t_ap(bk2d, NC6, 128))
            fc1b_sb = consts.tile([128, NH24], f32)
            nc.sync.dma_start(out=fc1b_sb, in_=t_ap(fc1b2d, NH24, 128))
            pb2_bc = consts.tile([128, C], f32)
            nc.sync.dma_start(out=pb2_bc, in_=bcast_ap(pb2, C))
            fc2b_bc = consts.tile([128, C], f32)
            nc.sync.dma_start(out=fc2b_bc, in_=bcast_ap(fc2b, C))
            if has_bv:
                bv_bc = consts.tile([128, C], f32)
                nc.sync.dma_start(out=bv_bc, in_=bcast_ap(bv1d, C))
            pw2T_sb = consts.tile([128, NC6, C], f32r)

            def load_x(b):
                tiles = []
                for t in range(NT):
                    r = ROWS[t]
                    xt = xp.tile([128, C], f32, tag="xt")
                    nc.gpsimd.dma_start(out=xt[:r], in_=x_in[b, t * 128:t * 128 + r, :])
                    tiles.append(xt)
                return tiles

            def load_wq():
                tiles = []
                for j in range(NC6):
                    wc = wchunk.tile([128, NC6, 128], f32r, tag="ws")
                    nc.sync.dma_start(
                        out=wc,
                        in_=wq[:, j * 128:(j + 1) * 128]
                        .rearrange("(a p) j -> p a j", p=128))
                    tiles.append(wc)
                return tiles

            def compute_ht(xts):
                tiles = []
                for t in range(NT):
                    r = ROWS[t]
                    xt = xts[t]
                    st = small.tile([128, 2, 6], f32, tag="bn")
                    xg = xt.rearrange("p (g d) -> p g d", g=2)
                    nc.vector.bn_stats(out=st[:r, 0], in_=xg[:r, 0])
                    nc.vector.bn_stats(out=st[:r, 1], in_=xg[:r, 1])
                    mv = small.tile([128, 2], f32, tag="mv")
                    nc.vector.bn_aggr(out=mv[:r], in_=st[:r])
                    sd = small.tile([128, 1], f32, tag="sd")
                    nc.scalar.activation(sd[:r], mv[:r, 1:2], AF.Sqrt, bias=eps_t[:r])
                    rc = small.tile([128, 1], f32, tag="rc")
                    nc.vector.reciprocal(rc[:r], sd[:r])
                    nmr = small.tile([128, 1], f32, tag="nmr")
                    nc.vector.tensor_scalar(
                        nmr[:r], mv[:r, 0:1], rc[:r], -1.0, ALU.mult, ALU.mult)
                    h = xp.tile([128, C], f32r, tag="xt")
                    nc.scalar.activation(h[:r], xt[:r], AF.Identity,
                                         bias=nmr[:r], scale=rc[:r])
                    tiles.append(h)
                return tiles

            xt_next = load_x(0)
            ht_next = compute_ht(xt_next)
            wq_next = None
            for b in range(BPC):
                # ---------------- Phase A: transpose ht -> hT ----------------
                ht = ht_next
                hT = []
                for c in range(NC6):
                    pA = ps.tile([128, 1024], f32r, tag="ps")
                    for t in range(NT):
                        r = ROWS[t] if ROWS[t] % 2 == 0 else ROWS[t] + 1
                        nc.tensor.transpose(
                            pA[:, t * 128:t * 128 + r],
                            ht[t][:r, c * 128:(c + 1) * 128], ident_r[:r, :r])
                    hc = p577.tile([128, N_PAD], f32r, tag="a577")
                    nc.scalar.copy(hc, pA[:, :N_PAD])
                    hT.append(hc)

                # ---------------- Phase B: qkv ----------------
                qT, kT = [], []
                wsrcs = ((wq, bq_sb, qT), (wk, bk_sb, kT)) if has_qkb \
                    else ((wq, bq_sb, qT),)
                for w_par, b_sb, dst in wsrcs:
                    for j in range(NC6):
                        if w_par is wq and wq_next is not None:
                            wc = wq_next[j]
                        else:
                            wc = wchunk.tile([128, NC6, 128], f32r, tag="ws")
                            nc.sync.dma_start(
                                out=wc,
                                in_=w_par[:, j * 128:(j + 1) * 128]
                                .rearrange("(a p) j -> p a j", p=128))
                        pQ = ps.tile([128, 1024], f32, tag="ps")
                        for c in range(NC6):
                            for (so, ln, po) in CHUNKS:
                                nc.tensor.matmul(
                                    pQ[:, po:po + ln], wc[:, c, :],
                                    hT[c][:, so:so + ln],
                                    start=(c == 0), stop=(c == NC6 - 1))
                        qj = p577.tile([128, N_PAD], f32r, tag="a577")
                        for (so, ln, po) in CHUNKS:
                            nc.scalar.activation(
                                qj[:, so:so + ln], pQ[:, po:po + ln],
                                AF.Identity, bias=b_sb[:, j:j + 1])
                        dst.append(qj)
                wv_sb = wvp.tile([128, NC6, C], f32r, tag="wv")
                for _c in range(NC6):
                    nc.sync.dma_start(
                        out=wv_sb[:, _c],
                        in_=wv[_c * 128:(_c + 1) * 128, :].bitcast(f32r))
                v = []
                for t in range(NT):
                    r = ROWS[t]
                    pV = ps.tile([128, 1024], f32, tag="ps")
                    for c in range(NC6):
                        nc.tensor.matmul(pV[:r, 0:512],
                                         hT[c][:, t * 128:t * 128 + r],
                                         wv_sb[:, c, 0:512],
                                         start=(c == 0), stop=(c == NC6 - 1))
                        nc.tensor.matmul(pV[:r, 512:768],
                                         hT[c][:, t * 128:t * 128 + r],
                                         wv_sb[:, c, 512:768],
                                         start=(c == 0), stop=(c == NC6 - 1))
                    vt = p768.tile([128, C], f32r, tag="a768")
                    if has_bv:
                        nc.vector.tensor_tensor(
                            out=vt[:r], in0=pV[:r, 0:768], in1=bv_bc[:r], op=ALU.add)
                    else:
                        nc.scalar.copy(vt[:r], pV[:r, 0:768])
                    v.append(vt)

                # ---------------- Phase C: S^T, exp, denom, r ----------------
                kTS = kT if has_qkb else hT
                PuT = []
                for m in range(NT):
                    rm = ROWS[m]
                    pS = ps.tile([128, 1024], f32, tag="ps")
                    for c in range(NC6):
                        for (so, ln, po) in CHUNKS:
                            nc.tensor.matmul(
                                pS[:rm, po:po + ln],
                                kTS[c][:, m * 128:m * 128 + rm],
                                qT[c][:, so:so + ln],
                                start=(c == 0), stop=(c == NC6 - 1))
                    pu = p577.tile([128, N_PAD], f32r, tag="a577")
                    for (so, ln, po) in CHUNKS:
                        nc.scalar.activation(
                            pu[:rm, so:so + ln], pS[:rm, po:po + ln],
                            AF.Exp, scale=scale)
                    PuT.append(pu)
                pD = ps.tile([128, 1024], f32, tag="ps")
                for m in range(NT):
                    rm = ROWS[m]
                    for (so, ln, po) in CHUNKS:
                        nc.tensor.matmul(
                            pD[0:1, po:po + ln], ones_col[:rm],
                            PuT[m][:rm, so:so + ln],
                            start=(m == 0), stop=(m == NT - 1))
                dsum = row577.tile([1, N_PAD], f32, tag="row")
                for (so, ln, po) in CHUNKS:
                    nc.scalar.copy(dsum[:, so:so + ln], pD[0:1, po:po + ln])
                # broadcast the raw denominator on PE first, then reciprocal on
                # DVE over all 128 partitions -- PE never waits on the recip.
                pR = ps.tile([128, 1024], f32, tag="ps")
                for (so, ln, po) in CHUNKS:
                    nc.tensor.matmul(pR[:, po:po + ln], ones_row,
                                     dsum[:, so:so + ln], start=True, stop=True)
                rbc = p577.tile([128, N_PAD], f32, tag="a577")
                for (so, ln, po) in CHUNKS:
                    nc.vector.reciprocal(rbc[:, so:so + ln], pR[:, po:po + ln])

                # token_attn: normalized attention row 0 = PuT[m][:, 0] * r[0]
                ta = small.tile([128, NT], f32, tag="ta")
                for m in range(NT):
                    nc.vector.tensor_copy(ta[:ROWS[m], m:m + 1], PuT[m][:ROWS[m], 0:1])
                nc.vector.tensor_scalar_mul(ta, ta, rbc[:, 0:1])
                nc.sync.dma_start(
                    out=bass.AP(tensor=tattn_d[:].tensor, offset=b * (N - 1),
                                ap=[[1, 127]]),
                    in_=ta[1:128, 0:1])
                nc.sync.dma_start(
                    out=bass.AP(tensor=tattn_d[:].tensor, offset=b * (N - 1) + 127,
                                ap=[[1, 128], [128, 3]]),
                    in_=ta[:, 1:4])
                nc.sync.dma_start(
                    out=bass.AP(tensor=tattn_d[:].tensor, offset=b * (N - 1) + 511,
                                ap=[[1, 65]]),
                    in_=ta[0:65, 4:5])

                if b == 0:
                    nc.sync.dma_start(
                        out=pw2T_sb,
                        in_=pw2T[:, :].rearrange("(a p) o -> p a o", p=128))
                # ---------------- Phase D: y^T = (v^T @ Pu^T) * r ----------------
                ydram = dpool.tile([C * N], f32, tag="yd")
                yd_w = ydram.rearrange("(a b) -> a b", b=N)    # [C, N] write view
                yd_r = ydram.rearrange("(a b) -> a b", b=C)    # [N, C] read view
                for c in range(NC6):
                    pY = ps.tile([128, 1024], f32, tag="ps")
                    for m in range(NT):
                        rm = ROWS[m]
                        for (so, ln, po) in CHUNKS:
                            nc.tensor.matmul(
                                pY[:, po:po + ln],
                                v[m][:rm, c * 128:(c + 1) * 128],
                                PuT[m][:rm, so:so + ln],
                                start=(m == 0), stop=(m == NT - 1))
                    yc = p577.tile([128, N_PAD], f32, tag="a577")
                    for (so, ln, po) in CHUNKS:
                        nc.vector.tensor_tensor(
                            out=yc[:, so:so + ln], in0=pY[:, po:po + ln],
                            in1=rbc[:, so:so + ln], op=ALU.mult)
                    nc.sync.dma_start(out=yd_w[c * 128:(c + 1) * 128, :], in_=yc[:, :N])

                # ---------------- Phase E: scramble read, transpose, proj ----------------
                ys = []
                for t in range(NT):
                    r = ROWS[t]
                    yst = p768.tile([128, C], f32r, tag="a768")
                    nc.sync.dma_start(out=yst[:r],
                                      in_=yd_r[t * 128:t * 128 + r, :].bitcast(f32r))
                    ys.append(yst)
                ysT = []
                for c in range(NC6):
                    pT2 = ps.tile([128, 1024], f32r, tag="ps")
                    for t in range(NT):
                        r = ROWS[t] if ROWS[t] % 2 == 0 else ROWS[t] + 1
                        nc.tensor.transpose(
                            pT2[:, t * 128:t * 128 + r],
                            ys[t][:r, c * 128:(c + 1) * 128], ident_r[:r, :r])
                    yTc = p577.tile([128, N_PAD], f32r, tag="a577")
                    nc.scalar.copy(yTc, pT2[:, :N_PAD])
                    ysT.append(yTc)
                x2 = []
                for t in range(NT):
                    r = ROWS[t]
                    pP = ps.tile([128, 1024], f32, tag="ps")
                    for c in range(NC6):
                        nc.tensor.matmul(pP[:r, 0:512],
                                         ysT[c][:, t * 128:t * 128 + r],
                                         pw2T_sb[:, c, 0:512],
                                         start=(c == 0), stop=(c == NC6 - 1))
                        nc.tensor.matmul(pP[:r, 512:768],
                                         ysT[c][:, t * 128:t * 128 + r],
                                         pw2T_sb[:, c, 512:768],
                                         start=(c == 0), stop=(c == NC6 - 1))
                    x2t = p768.tile([128, C], f32, tag="a768")
                    nc.vector.tensor_tensor(
                        out=x2t[:r], in0=pP[:r, 0:768], in1=pb2_bc[:r], op=ALU.add)
                    x2.append(x2t)

                # ---------------- Phase F: LN2 + transpose -> mT ----------------
                mtk = []
                for t in range(NT):
                    r = ROWS[t]
                    st = small.tile([128, 2, 6], f32, tag="bn")
                    xg = x2[t].rearrange("p (g d) -> p g d", g=2)
                    nc.vector.bn_stats(out=st[:r, 0], in_=xg[:r, 0])
                    nc.vector.bn_stats(out=st[:r, 1], in_=xg[:r, 1])
                    mv = small.tile([128, 2], f32, tag="mv")
                    nc.vector.bn_aggr(out=mv[:r], in_=st[:r])
                    sd = small.tile([128, 1], f32, tag="sd")
                    nc.scalar.activation(sd[:r], mv[:r, 1:2], AF.Sqrt, bias=eps_t[:r])
                    rc = small.tile([128, 1], f32, tag="rc")
                    nc.vector.reciprocal(rc[:r], sd[:r])
                    nmr = small.tile([128, 1], f32, tag="nmr")
                    nc.vector.tensor_scalar(
                        nmr[:r], mv[:r, 0:1], rc[:r], -1.0, ALU.mult, ALU.mult)
                    mt = p768.tile([128, C], f32r, tag="a768")
                    nc.scalar.activation(mt[:r], x2[t][:r], AF.Identity,
                                         bias=nmr[:r], scale=rc[:r])
                    mtk.append(mt)
                mT = []
                for c in range(NC6):
                    pT3 = ps.tile([128, 1024], f32r, tag="ps")
                    for t in range(NT):
                        r = ROWS[t] if ROWS[t] % 2 == 0 else ROWS[t] + 1
                        nc.tensor.transpose(
                            pT3[:, t * 128:t * 128 + r],
                            mtk[t][:r, c * 128:(c + 1) * 128], ident_r[:r, :r])
                    mc = p577.tile([128, N_PAD], f32r, tag="a577")
                    nc.scalar.copy(mc, pT3[:, :N_PAD])
                    mT.append(mc)

                # ---------------- Phase G: fc1 chunks + fc2 ----------------
                if b + 1 < BPC:
                    xt_next = load_x(b + 1)
                    ht_next = compute_ht(xt_next)
                g_sb = [None] * NT
                NGRP = 3
                GSZ = NH24 // NGRP
                for half in range(NGRP):
                    fT = []
                    for hl in range(GSZ):
                        hh = half * GSZ + hl
                        f1c = wchunk.tile([128, NC6, 128], f32r, tag="ws")
                        nc.sync.dma_start(
                            out=f1c,
                            in_=fc1wT[:, hh * 128:(hh + 1) * 128]
                            .rearrange("(a p) j -> p a j", p=128))
                        pF = ps.tile([128, 1024], f32, tag="ps")
                        for c in range(NC6):
                            for (so, ln, po) in CHUNKS:
                                nc.tensor.matmul(
                                    pF[:, po:po + ln], f1c[:, c, :],
                                    mT[c][:, so:so + ln],
                                    start=(c == 0), stop=(c == NC6 - 1))
                        ft = p577.tile([128, N_PAD], f32r, tag="a577")
                        for (so, ln, po) in CHUNKS:
                            nc.scalar.activation(
                                ft[:, so:so + ln], pF[:, po:po + ln],
                                AF.Gelu, bias=fc1b_sb[:, hh:hh + 1])
                        fT.append(ft)
                    f2c = []
                    for hl in range(GSZ):
                        hh = half * GSZ + hl
                        fc = fc2c_pool.tile([128, C], f32r, tag="f2c")
                        nc.sync.dma_start(out=fc, in_=fc2wT[hh * 128:(hh + 1) * 128, :])
                        f2c.append(fc)
                    if half == NGRP - 1:
                        wq_next = load_wq() if b + 1 < BPC else None
                    for t in range(NT):
                        r = ROWS[t]
                        pG = ps.tile([128, 1024], f32, tag="ps")
                        for hl in range(GSZ):
                            nc.tensor.matmul(pG[:r, 0:512],
                                             fT[hl][:, t * 128:t * 128 + r],
                                             f2c[hl][:, 0:512],
                                             start=(hl == 0), stop=(hl == GSZ - 1))
                            nc.tensor.matmul(pG[:r, 512:768],
                                             fT[hl][:, t * 128:t * 128 + r],
                                             f2c[hl][:, 512:768],
                                             start=(hl == 0), stop=(hl == GSZ - 1))
                        if half == 0:
                            gt = p768.tile([128, C], f32, tag="a768")
                            nc.vector.tensor_tensor(
                                out=gt[:r], in0=pG[:r, 0:768], in1=fc2b_bc[:r],
                                op=ALU.add)
                            g_sb[t] = gt
                        elif half < NGRP - 1:
                            nc.vector.tensor_tensor(
                                out=g_sb[t][:r], in0=pG[:r, 0:768],
                                in1=g_sb[t][:r], op=ALU.add)
                        else:
                            nc.vector.tensor_tensor(
                                out=g_sb[t][:r], in0=pG[:r, 0:768],
                                in1=g_sb[t][:r], op=ALU.add)
                            ot = p768.tile([128, C], f32, tag="a768")
                            nc.vector.tensor_tensor(
                                out=ot[:r], in0=g_sb[t][:r], in1=x2[t][:r],
                                op=ALU.add)
                            nc.sync.dma_start(
                                out=out_d[b, t * 128:t * 128 + r, :], in_=ot[:r])

    _legalize_sync(nc, mybir)
    return nc


_PROG_CACHE = {}


def _get_program(has_bv, has_qkb):
    key = (has_bv, has_qkb)
    if key not in _PROG_CACHE:
        _PROG_CACHE[key] = _build_program(has_bv, has_qkb)
    return _PROG_CACHE[key]


def _prep_host(x, norm1_w, norm1_b, qkv_w, proj_w, proj_b,
               norm2_w, norm2_b, fc1_w, fc1_b, fc2_w, fc2_b):
    f = np.float32
    x = np.ascontiguousarray(np.asarray(x, f))
    w1 = np.asarray(norm1_w, f); b1 = np.asarray(norm1_b, f)
    w2 = np.asarray(norm2_w, f); b2 = np.asarray(norm2_b, f)
    qkv_w = np.asarray(qkv_w, f); proj_w = np.asarray(proj_w, f)
    fc1_w = np.asarray(fc1_w, f); fc2_w = np.asarray(fc2_w, f)

    qkv_w_eff = qkv_w * w1[None, :]
    qkv_b_eff = qkv_w @ b1
    wqT = np.ascontiguousarray(qkv_w_eff[0:C].T)
    wkT = np.ascontiguousarray(qkv_w_eff[C:2 * C].T)
    wvT = np.ascontiguousarray(qkv_w_eff[2 * C:3 * C].T)
    bq = np.ascontiguousarray(qkv_b_eff[0:C].reshape(NC6, 128))
    bk = np.ascontiguousarray(qkv_b_eff[C:2 * C].reshape(NC6, 128))
    bv = np.ascontiguousarray(qkv_b_eff[2 * C:3 * C])
    pw2T = np.ascontiguousarray((2.0 * proj_w).T)
    pb2 = np.ascontiguousarray(2.0 * np.asarray(proj_b, f))
    fc1wT = np.ascontiguousarray((fc1_w * w2[None, :]).T)
    fc1b_eff = np.ascontiguousarray(
        (np.asarray(fc1_b, f) + fc1_w @ b2).reshape(NH24, 128))
    fc2wT = np.ascontiguousarray(fc2_w.T)
    fc2b = np.ascontiguousarray(np.asarray(fc2_b, f))
    return x, wqT, wkT, wvT, bq, bk, bv, pw2T, pb2, fc1wT, fc1b_eff, fc2wT, fc2b


def _run(inputs, trace=False):
    from concourse.bass_utils import run_bass_kernel_spmd

    (x, wqT, wkT, wvT, bq, bk, bv, pw2T, pb2, fc1wT, fc1b_eff,
     fc2wT, fc2b) = _prep_host(**inputs)
    has_bv = bool(np.any(bv))
    has_qkb = bool(np.any(bq)) or bool(np.any(bk))
    if not has_qkb:
        # S = q k^T = h (Wq^T Wk) h^T: stream A = Wq^T Wk instead of Wq, Wk
        # (wqT/wkT here are already transposed: wqT = Wq^T as [c, j]).
        A = (wqT.astype(np.float64) @ wkT.T.astype(np.float64)).astype(np.float32)
        wqT = np.ascontiguousarray(A)
    nc = _get_program(has_bv, has_qkb)

    in_maps = []
    for core in range(N_CORES):
        in_maps.append({
            "x": np.ascontiguousarray(x[core * BPC:(core + 1) * BPC]),
            "wq": wqT, "wk": wkT, "wv": wvT, "pw2T": pw2T,
            "fc1wT": fc1wT, "fc2wT": fc2wT,
            "bq2d": bq, "bk2d": bk, "fc1b2d": fc1b_eff,
            "pb2": pb2, "fc2b": fc2b, "bv1d": bv,
        })
    res = run_bass_kernel_spmd(nc, in_maps, core_ids=list(range(N_CORES)),
                               trace=trace)
    out = np.concatenate([res.results[i]["out"] for i in range(N_CORES)], axis=0)
    ta = np.concatenate([res.results[i]["tattn"] for i in range(N_CORES)], axis=0)
    asz = int((N - 1) ** 0.5)
    token_attn = ta.reshape(B, 1, asz, asz).astype(np.float32)
    return (out.astype(np.float32), token_attn), res


def kernel(**inputs):
    (out, token_attn), _ = _run(inputs, trace=False)
    return (out, token_attn)
